# revision 35
# baseline (speedup 1.0000x reference)
"""Ragged cross-attention pooling kernel for Trainium2 (8 NeuronCores, SPMD).

Math (per pair, direction "A attends over B"):
    qa = (A @ Wq*scale + bq*scale)      [la, INNER]
    kb =  B @ Wk + bk                   [lb, INNER]
    s  = qa @ kb^T                      [la, lb]
    p  = exp(s)               (no max-subtraction needed: |s| <~ 6)
    den[q] = sum_k p[q, k]  (pad-corrected: all pad cols share p[:, -1])
    gcol[q] = valid(q) / (la * den[q])
    w[k] = sum_q gcol[q] p[q, k]        <- collapses the mean over queries
    emb  = (w^T B) @ Wv + bv            <- collapses attn@V and the V projection

v2: A/B pre-transposed ON HOST (no on-chip transposes), all matmul inputs
bf16 (1 cyc/row at any moving size), single wide exp per query tile into a
2-bank PSUM tile, w row->col via SBUF-to-SBUF scatter DMA, final E computed
as E^T = U^T Wv with 16-wide stationary.

Distribution: 64 pairs -> 8 slots x 8 cores (one shared SPMD program, shapes
fixed per slot to the max over cores; pairs bin-packed by length so padding is
small).
"""

import os
import sys

sys.path.insert(0, "/opt/trn_rl_repo")

import numpy as np

B, LA, LB, DIM, INNER, OUTER = 64, 1024, 1024, 640, 256, 1024
NCORES, NSLOTS, P = 8, 8, 128
SCALE = 1.0 / np.sqrt(INNER)
DT = DIM // P  # 5 d-chunks
MI = INNER // P  # 2 inner-chunks

LAST_EXEC_TIME_NS = None


def _chunks(total, cap=512):
    out, off = [], 0
    while off < total:
        c = min(cap, total - off)
        out.append((off, c))
        off += c
    return out


def _plan(la_all, lb_all):
    """Assign pairs to (slot, core); returns swap flags, groups, slot tile shapes."""
    la = np.asarray(la_all, np.int64)
    lb = np.asarray(lb_all, np.int64)
    swap = lb > la
    qa = np.where(swap, lb, la)  # kernel A-side length (>= B-side)
    qb = np.where(swap, la, lb)
    at = -(-qa // P)
    bt = -(-qb // P)
    order = np.argsort(-(at * 1024 + bt), kind="stable")
    groups = [list(order[s * NCORES:(s + 1) * NCORES]) for s in range(NSLOTS)]
    C1, C2 = 1430.0, 430.0

    def gcost(g):
        ma = max(at[i] for i in g)
        mb = max(bt[i] for i in g)
        return C1 * (ma + mb) + C2 * ma * mb

    rng = np.random.default_rng(0)
    cost = [gcost(g) for g in groups]
    s1s = rng.integers(0, NSLOTS, 30000)
    s2s = rng.integers(0, NSLOTS, 30000)
    i1s = rng.integers(0, NCORES, 30000)
    i2s = rng.integers(0, NCORES, 30000)
    for s1, s2, i1, i2 in zip(s1s, s2s, i1s, i2s):
        if s1 == s2:
            continue
        g1 = groups[s1][:]
        g2 = groups[s2][:]
        g1[i1], g2[i2] = groups[s2][i2], groups[s1][i1]
        n1, n2 = gcost(g1), gcost(g2)
        if n1 + n2 < cost[s1] + cost[s2] - 1e-9:
            groups[s1], groups[s2] = g1, g2
            cost[s1], cost[s2] = n1, n2
    slot_at = [max(at[i] for i in g) for g in groups]
    slot_bt = [max(bt[i] for i in g) for g in groups]
    # run small slots first: minimizes the pipeline-fill bubble
    sorder = sorted(range(NSLOTS), key=lambda s: cost[s])
    groups = [groups[s] for s in sorder]
    slot_at = [slot_at[s] for s in sorder]
    slot_bt = [slot_bt[s] for s in sorder]
    return swap, qa, qb, groups, slot_at, slot_bt


def _build_program(slot_at, slot_bt):
    import concourse.bass as bass  # noqa: F401
    import concourse.mybir as mybir
    import concourse.tile as tile
    from concourse.tile import add_dep_helper
    from concourse import bacc

    F32 = mybir.dt.float32
    F32R = mybir.dt.float32r
    BF16 = mybir.dt.bfloat16
    FP8 = mybir.dt.float8e4
    DR = mybir.MatmulPerfMode.DoubleRow
    Exp = mybir.ActivationFunctionType.Exp
    Ident = mybir.ActivationFunctionType.Identity

    tot_at = sum(slot_at)
    tot_bt = sum(slot_bt)
    cum_at = np.concatenate([[0], np.cumsum(slot_at)]).astype(int)
    cum_bt = np.concatenate([[0], np.cumsum(slot_bt)]).astype(int)

    nc = bacc.Bacc("TRN2", target_bir_lowering=False, debug=False,
                   num_devices=NCORES)

    tot = tot_at + tot_bt
    # natural layout (row-tiled; per slot A-rows then B-rows) for u = w^T B
    nat_d = nc.dram_tensor("nat", [tot * P, DIM], BF16, kind="ExternalInput")
    # host-transposed layout [dpart, dt, seq] (per slot A-cols then B-cols)
    tr_d = nc.dram_tensor("tr", [P, DT, tot * P], BF16, kind="ExternalInput")
    # all small per-core constants packed into one tensor:
    # [gs_a | gs_b | npa | npb | bqs | bk] along the free dim
    NSM = tot_at + tot_bt + NSLOTS + NSLOTS + MI + MI
    sm_d = nc.dram_tensor("sm", [P, NSM], F32, kind="ExternalInput")
    wq_d = nc.dram_tensor("wq", [P, DT, INNER], BF16, kind="ExternalInput")
    wk_d = nc.dram_tensor("wk", [P, DT, INNER], BF16, kind="ExternalInput")
    wv_d = nc.dram_tensor("wv", [P, DT, OUTER], BF16, kind="ExternalInput")
    bvb_d = nc.dram_tensor("bvb", [2 * NSLOTS, OUTER], F32,
                           kind="ExternalInput")
    idr_d = nc.dram_tensor("idr", [P, P], F32R, kind="ExternalInput")
    emb_d = nc.dram_tensor("emb", [2 * NSLOTS, OUTER], F32,
                           kind="ExternalOutput")
    # DRAM bounce buffer for the w row->col partition scatter
    wsc_d = nc.dram_tensor("wsc", [2 * NSLOTS, NCORES * P], BF16,
                           kind="ExternalOutput")
    cum = np.concatenate([[0], np.cumsum(
        [slot_at[s] + slot_bt[s] for s in range(NSLOTS)])]).astype(int)

    with tile.TileContext(nc) as tc:
        with (
            tc.tile_pool(name="const", bufs=1) as cpool,
            tc.tile_pool(name="anat", bufs=2) as apool,
            tc.tile_pool(name="atr", bufs=2) as atpool,
            tc.tile_pool(name="proj", bufs=2) as ppool,
            tc.tile_pool(name="pexp", bufs=2) as epool,
            tc.tile_pool(name="small", bufs=3) as spool,
            tc.tile_pool(name="late", bufs=2) as lpool,
            tc.tile_pool(name="psB", bufs=3, space="PSUM") as psB,
            tc.tile_pool(name="psW", bufs=1, space="PSUM") as psW,
        ):
            # ---- constants ----
            wq_sb = cpool.tile([P, DT, INNER], BF16, tag="wq")
            wk_sb = cpool.tile([P, DT, INNER], BF16, tag="wk")
            wv_sb = cpool.tile([P, DT, OUTER], BF16, tag="wv")
            bvb_sb = cpool.tile([2 * NSLOTS, OUTER], F32, tag="bvb")
            idr_sb = cpool.tile([P, P], F32R, tag="idr")
            sm_sb = cpool.tile([P, NSM], F32, tag="sm")
            # column offsets into sm_sb: [gs_a | gs_b | npa | npb | bqs | bk]
            GA, GB = 0, tot_at
            NPA, NPB = tot_at + tot_bt, tot_at + tot_bt + NSLOTS
            BQ, BK = NSM - 2 * MI, NSM - MI
            urows_sb = cpool.tile([2 * NSLOTS, DIM], F32R, tag="urows")

            ev = 0  # evac engine alternator
            for s in range(NSLOTS):
                at_s, bt_s = int(slot_at[s]), int(slot_bt[s])
                pla, plb = at_s * P, bt_s * P
                nt = at_s + bt_s
                # ---- loads: transposed first (projections), natural later ----
                tr_sb = atpool.tile([P, DT, nt * P], BF16, tag="tr")
                nc.sync.dma_start(
                    tr_sb[:], tr_d[:, :, cum[s] * P:cum[s + 1] * P])
                if s == 0:
                    # slot-0 inputs are already in flight; now the rest
                    nc.sync.dma_start(sm_sb[:], sm_d[:])
                    nc.sync.dma_start(wq_sb[:], wq_d[:])
                    nc.sync.dma_start(wk_sb[:], wk_d[:])
                nat_sb = apool.tile([P, nt, DIM], BF16, tag="nat")
                nc.sync.dma_start(
                    nat_sb[:], nat_d[cum[s] * P:cum[s + 1] * P, :]
                    .rearrange("(t p) d -> p t d", p=P))

                # ---- projections (order: dir-A deps first) ----
                # q/k stored fp8-e4m3: scores then run one DoubleRow matmul
                # per chunk (0.5 cyc/row, all 256 contraction at once)
                qaT = ppool.tile([P, MI, pla], FP8, tag="qaT")
                kaT = ppool.tile([P, MI, pla], FP8, tag="kaT")
                qbT = ppool.tile([P, MI, plb], FP8, tag="qbT")
                kbT = ppool.tile([P, MI, plb], FP8, tag="kbT")
                for dst, soff, pl, w_sb, bo in (
                        (qaT, 0, pla, wq_sb, BQ),
                        (kbT, pla, plb, wk_sb, BK),
                        (kaT, 0, pla, wk_sb, BK),
                        (qbT, pla, plb, wq_sb, BQ)):
                    for m in range(MI):
                        pp = psB.tile([P, 1024], F32, tag="big")
                        for kt in range(DT):
                            for noff, nlen in _chunks(pl):
                                nc.tensor.matmul(
                                    pp[:, noff:noff + nlen],
                                    w_sb[:, kt, m * P:(m + 1) * P],
                                    tr_sb[:, kt,
                                          soff + noff:soff + noff + nlen],
                                    start=(kt == 0), stop=(kt == DT - 1))
                        if ev % 2 == 0:
                            nc.vector.tensor_scalar_add(
                                dst[:, m, :], pp[:, :pl],
                                sm_sb[:, bo + m, None])
                        else:
                            nc.scalar.activation(
                                dst[:, m, :], pp[:, :pl],
                                Ident, bias=sm_sb[:, bo + m, None], scale=1.0)
                        ev += 1

                # ---- attention directions ----
                for dr in range(2):
                    if dr == 0:  # A queries over B keys
                        QT, KT, nq, nk = qaT, kbT, at_s, bt_s
                        g_off = GA + cum_at[s]
                        np_off = NPB + s
                        koff = at_s  # B rows sit after A rows in nat_sb
                    else:
                        QT, KT, nq, nk = qbT, kaT, bt_s, at_s
                        g_off = GB + cum_bt[s]
                        np_off = NPA + s
                        koff = 0
                    plk = nk * P
                    kchunks = _chunks(plk)
                    wr = [psW.tile([1, cl], F32, tag=f"wr{ci}",
                                   name=f"wr{ci}")
                          for ci, (co, cl) in enumerate(kchunks)]
                    for qt in range(nq):
                        sc = psB.tile([P, 1024], F32, tag="big")
                        for co, cl in kchunks:
                            nc.tensor.matmul(
                                sc[:, co:co + cl],
                                QT[:, :, qt * P:(qt + 1) * P],
                                KT[:, :, co:co + cl],
                                start=True, stop=True, perf_mode=DR)
                        # pad contribution npad*p_pad as exp(s_pad+ln(npad))
                        # in f32 (bf16 p_pad would amplify through the
                        # den - npad*p_pad cancellation); np_sb holds ln(npad).
                        # Emitted BEFORE the big exp so the vector chain can
                        # overlap it.
                        pc = spool.tile([P, 1], F32, tag="pc")
                        nc.scalar.activation(
                            pc[:], sc[:, plk - 1:plk], Exp,
                            bias=sm_sb[:, np_off, None], scale=SCALE)
                        den = spool.tile([P, 1], F32, tag="den")
                        p_sb = epool.tile([P, plk], BF16, tag="p_sb")
                        nc.scalar.activation(
                            p_sb[:], sc[:, :plk], Exp,
                            bias=0.0, scale=SCALE, accum_out=den[:])
                        # den_f = pad - den  (= -true_den; g is negated on
                        # host so gcol comes out positive)
                        denf = spool.tile([P, 1], F32, tag="denf")
                        nc.vector.tensor_sub(denf[:], pc[:], den[:])
                        rec = spool.tile([P, 1], F32, tag="rec")
                        nc.vector.reciprocal(rec[:], denf[:])
                        gcol = spool.tile([P, 1], BF16, tag="gcol")
                        nc.vector.tensor_mul(gcol[:], rec[:],
                                             sm_sb[:, g_off + qt, None])
                        for ci, (co, cl) in enumerate(kchunks):
                            nc.tensor.matmul(
                                wr[ci][:], gcol[:], p_sb[:, co:co + cl],
                                start=(qt == 0), stop=(qt == nq - 1))
                    # w row -> w col: partition scatter via a DRAM bounce
                    # (keeps the PE free of per-tile transpose matmuls)
                    wrow = lpool.tile([1, plk], BF16, tag="wrow")
                    for ci, (co, cl) in enumerate(kchunks):
                        nc.scalar.copy(wrow[0:1, co:co + cl], wr[ci][:])
                    r = 2 * s + dr
                    d_out = nc.sync.dma_start(
                        wsc_d[r:r + 1, :plk], wrow[:])
                    wcol = lpool.tile([P, nk], BF16, tag="wcol")
                    d_in = nc.sync.dma_start(
                        wcol[:],
                        wsc_d[r:r + 1, :plk].rearrange(
                            "o (t p) -> (o p) t", p=P))
                    add_dep_helper(d_in.ins, d_out.ins, reason="wsc RAW")
                    # u row = w^T @ Knat
                    ur = psB.tile([P, 1024], F32, tag="big")
                    for noff, nlen in _chunks(DIM):
                        for kt in range(nk):
                            nc.tensor.matmul(
                                ur[0:1, noff:noff + nlen],
                                wcol[:, kt:kt + 1],
                                nat_sb[:, koff + kt, noff:noff + nlen],
                                start=(kt == 0), stop=(kt == nk - 1))
                    ursb = lpool.tile([1, DIM], F32R, tag="ursb")
                    nc.scalar.copy(ursb[:], ur[0:1, :DIM])
                    nc.sync.dma_start(
                        urows_sb[2 * s + dr:2 * s + dr + 1, :], ursb[:])

            # ---- final: E^T = U^T Wv + bv ----
            for sb, d in ((wv_sb, wv_d), (bvb_sb, bvb_d), (idr_sb, idr_d)):
                nc.sync.dma_start(sb[:], d[:])
            u_sb = cpool.tile([P, DT, 2 * NSLOTS], BF16, tag="usb")
            for dt in range(DT):
                ut = psB.tile([P, 1024], F32, tag="big")
                nc.tensor.matmul(
                    ut[:, :2 * NSLOTS],
                    urows_sb[:, dt * P:(dt + 1) * P],
                    idr_sb[0:2 * NSLOTS, 0:2 * NSLOTS],
                    start=True, stop=True)
                nc.vector.tensor_copy(u_sb[:, dt, :], ut[:, :2 * NSLOTS])
            eT = psB.tile([P, 1024], F32, tag="big")
            for noff, nlen in _chunks(OUTER):
                for dt in range(DT):
                    nc.tensor.matmul(
                        eT[0:2 * NSLOTS, noff:noff + nlen],
                        u_sb[:, dt, :],
                        wv_sb[:, dt, noff:noff + nlen],
                        start=(dt == 0), stop=(dt == DT - 1))
            e_sb = cpool.tile([2 * NSLOTS, OUTER], F32, tag="esb")
            nc.vector.tensor_add(e_sb[:], eT[0:2 * NSLOTS, :], bvb_sb[:])
            nc.sync.dma_start(emb_d[:], e_sb[:])

    nc.compile()
    return nc


def _install_profhook():
    import contextlib
    import ctypes
    import types

    import antenv

    if not hasattr(antenv, "axon_hooks"):
        mod = types.ModuleType("antenv.axon_hooks")
        mod._hook = None

        def _set(h):
            mod._hook = h

        def _get():
            return mod._hook

        mod.set_axon_ntff_profile_hook = _set
        mod.get_axon_ntff_profile_hook = _get
        sys.modules["antenv.axon_hooks"] = mod
        antenv.axon_hooks = mod
    from antenv.axon_hooks import set_axon_ntff_profile_hook
    so_path = "/opt/axon/libaxon_pjrt.so"
    if not os.path.exists(so_path):
        return False
    lib = ctypes.CDLL(so_path)
    if not hasattr(lib, "axon_start_nrt_profile"):
        return False
    lib.axon_start_nrt_profile.argtypes = [ctypes.POINTER(ctypes.c_int64),
                                           ctypes.c_size_t]
    lib.axon_start_nrt_profile.restype = ctypes.c_int64
    lib.axon_stop_nrt_profile.argtypes = [ctypes.c_char_p]
    lib.axon_stop_nrt_profile.restype = ctypes.c_int64

    @contextlib.contextmanager
    def _hook(output_dir, device_ids):
        import jax

        jax.devices()
        if device_ids:
            ids = (ctypes.c_int64 * len(device_ids))(*device_ids)
            rc = lib.axon_start_nrt_profile(ids, len(device_ids))
        else:
            rc = lib.axon_start_nrt_profile(None, 0)
        if rc != 0:
            raise RuntimeError(f"axon_start_nrt_profile rc={rc}")
        try:
            yield
        finally:
            n = lib.axon_stop_nrt_profile(str(output_dir).encode())
            print(f"profile: {n} file(s) written to {output_dir}",
                  file=sys.stderr)

    set_axon_ntff_profile_hook(_hook)
    return True


def kernel(a_pad, b_pad, len_a, len_b, Wq, bq, Wk, bk, Wv, bv):
    global LAST_EXEC_TIME_NS
    import ml_dtypes
    BF = ml_dtypes.bfloat16

    a_pad = np.ascontiguousarray(np.asarray(a_pad, np.float32))
    b_pad = np.ascontiguousarray(np.asarray(b_pad, np.float32))
    len_a = np.asarray(len_a, np.int32)
    len_b = np.asarray(len_b, np.int32)
    Wq = np.asarray(Wq, np.float32)
    Wk = np.asarray(Wk, np.float32)
    Wv = np.asarray(Wv, np.float32)
    bq = np.asarray(bq, np.float32)
    bk = np.asarray(bk, np.float32)
    bv = np.asarray(bv, np.float32)

    swap, qa_len, qb_len, groups, slot_at, slot_bt = _plan(len_a, len_b)
    tot_at, tot_bt = sum(slot_at), sum(slot_bt)
    cum_at = np.concatenate([[0], np.cumsum(slot_at)]).astype(int)
    cum_bt = np.concatenate([[0], np.cumsum(slot_bt)]).astype(int)
    tot = tot_at + tot_bt
    cum = np.concatenate([[0], np.cumsum(
        [slot_at[s] + slot_bt[s] for s in range(NSLOTS)])]).astype(int)
    NSM = tot_at + tot_bt + 2 * NSLOTS + 2 * MI

    # ---- shared (per-core-identical) inputs ----
    # 1/sqrt(INNER) applied via the exp activation's scale argument, so q
    # stays ~N(0,1) for fp8 storage
    wq_h = Wq.reshape(DT, P, INNER).transpose(1, 0, 2).astype(BF)
    wk_h = Wk.reshape(DT, P, INNER).transpose(1, 0, 2).astype(BF)
    wv_h = Wv.reshape(DT, P, OUTER).transpose(1, 0, 2).astype(BF)
    bqs_h = bq.reshape(MI, P).T.copy()
    bk_h = bk.reshape(MI, P).T.copy()
    bvb_h = np.broadcast_to(bv, (2 * NSLOTS, OUTER)).copy()
    idr_h = np.eye(P, dtype=np.float32)

    a16 = a_pad.astype(BF)
    b16 = b_pad.astype(BF)

    # ---- per-core inputs ----
    in_maps = []
    for c in range(NCORES):
        nat = np.zeros((tot * P, DIM), BF)
        tr = np.zeros((P, DT, tot * P), BF)
        sm = np.zeros((P, NSM), np.float32)
        gs_a = sm[:, 0:tot_at]
        gs_b = sm[:, tot_at:tot_at + tot_bt]
        npa = sm[:, tot_at + tot_bt:tot_at + tot_bt + NSLOTS]
        npb = sm[:, tot_at + tot_bt + NSLOTS:tot_at + tot_bt + 2 * NSLOTS]
        sm[:, NSM - 2 * MI:NSM - MI] = bqs_h
        sm[:, NSM - MI:NSM] = bk_h
        for s in range(NSLOTS):
            i = groups[s][c]
            la_i, lb_i = int(qa_len[i]), int(qb_len[i])
            A = b16[i] if swap[i] else a16[i]
            Bm = a16[i] if swap[i] else b16[i]
            ao = cum[s] * P                  # A rows/cols at slot start
            bo = (cum[s] + slot_at[s]) * P   # B rows/cols after A's
            nat[ao:ao + la_i] = A[:la_i]
            nat[bo:bo + lb_i] = Bm[:lb_i]
            # transposed layout: [dpart, dt, seq]
            tr[:, :, ao:ao + la_i] = \
                A[:la_i].T.reshape(DT, P, la_i).transpose(1, 0, 2)
            tr[:, :, bo:bo + lb_i] = \
                Bm[:lb_i].T.reshape(DT, P, lb_i).transpose(1, 0, 2)
            # g columns NEGATED (sign trick pairs with den_f = pc - den)
            ga = np.zeros(slot_at[s] * P, np.float32)
            ga[:la_i] = -1.0 / la_i
            gs_a[:, cum_at[s]:cum_at[s] + slot_at[s]] = \
                ga.reshape(slot_at[s], P).T
            gb = np.zeros(slot_bt[s] * P, np.float32)
            gb[:lb_i] = -1.0 / lb_i
            gs_b[:, cum_bt[s]:cum_bt[s] + slot_bt[s]] = \
                gb.reshape(slot_bt[s], P).T
            na_i = slot_at[s] * P - la_i
            nb_i = slot_bt[s] * P - lb_i
            npa[:, s] = np.log(na_i) if na_i > 0 else -1e30
            npb[:, s] = np.log(nb_i) if nb_i > 0 else -1e30
        in_maps.append({
            "nat": nat, "tr": tr, "sm": sm,
            "wq": wq_h, "wk": wk_h, "wv": wv_h,
            "bvb": bvb_h, "idr": idr_h,
        })

    nc = _build_program(slot_at, slot_bt)

    from concourse.bass_utils import run_bass_kernel_spmd

    trace = os.environ.get("BASS_KERNEL_TRACE", "0") == "1"
    if trace:
        _install_profhook()
    res = run_bass_kernel_spmd(nc, in_maps, list(range(NCORES)), trace=trace)
    LAST_EXEC_TIME_NS = res.exec_time_ns

    emb_a = np.zeros((B, OUTER), np.float32)
    emb_b = np.zeros((B, OUTER), np.float32)
    for c in range(NCORES):
        e = np.asarray(res.results[c]["emb"], np.float32)
        for s in range(NSLOTS):
            i = groups[s][c]
            ea, eb = e[2 * s], e[2 * s + 1]  # A-queries, B-queries
            if swap[i]:
                emb_a[i], emb_b[i] = eb, ea
            else:
                emb_a[i], emb_b[i] = ea, eb
    return emb_a, emb_b


# revision 37
# speedup vs baseline: 1.2929x; 1.2929x over previous
"""Ragged cross-attention pooling kernel for Trainium2 (8 NeuronCores, SPMD).

Math (per pair, direction "A attends over B"):
    qa = (A @ Wq*scale + bq*scale)      [la, INNER]
    kb =  B @ Wk + bk                   [lb, INNER]
    s  = qa @ kb^T                      [la, lb]
    p  = exp(s)               (no max-subtraction needed: |s| <~ 6)
    den[q] = sum_k p[q, k]  (pad-corrected: all pad cols share p[:, -1])
    gcol[q] = valid(q) / (la * den[q])
    w[k] = sum_q gcol[q] p[q, k]        <- collapses the mean over queries
    emb  = (w^T B) @ Wv + bv            <- collapses attn@V and the V projection

v2: A/B pre-transposed ON HOST (no on-chip transposes), all matmul inputs
bf16 (1 cyc/row at any moving size), single wide exp per query tile into a
2-bank PSUM tile, w row->col via SBUF-to-SBUF scatter DMA, final E computed
as E^T = U^T Wv with 16-wide stationary.

Distribution: 64 pairs -> 8 slots x 8 cores (one shared SPMD program, shapes
fixed per slot to the max over cores; pairs bin-packed by length so padding is
small).
"""

import os
import sys

sys.path.insert(0, "/opt/trn_rl_repo")

import numpy as np

B, LA, LB, DIM, INNER, OUTER = 64, 1024, 1024, 640, 256, 1024
NCORES, NSLOTS, P = 8, 8, 128
SCALE = 1.0 / np.sqrt(INNER)
DT = DIM // P  # 5 d-chunks
MI = INNER // P  # 2 inner-chunks

LAST_EXEC_TIME_NS = None


def _chunks(total, cap=512):
    out, off = [], 0
    while off < total:
        c = min(cap, total - off)
        out.append((off, c))
        off += c
    return out


def _plan(la_all, lb_all):
    """Assign pairs to (slot, core); returns swap flags, groups, slot tile shapes."""
    la = np.asarray(la_all, np.int64)
    lb = np.asarray(lb_all, np.int64)
    swap = lb > la
    qa = np.where(swap, lb, la)  # kernel A-side length (>= B-side)
    qb = np.where(swap, la, lb)
    at = -(-qa // P)
    bt = -(-qb // P)
    order = np.argsort(-(at * 1024 + bt), kind="stable")
    groups = [list(order[s * NCORES:(s + 1) * NCORES]) for s in range(NSLOTS)]
    C1, C2 = 1430.0, 430.0

    def gcost(g):
        ma = max(at[i] for i in g)
        mb = max(bt[i] for i in g)
        return C1 * (ma + mb) + C2 * ma * mb

    rng = np.random.default_rng(0)
    cost = [gcost(g) for g in groups]
    s1s = rng.integers(0, NSLOTS, 30000)
    s2s = rng.integers(0, NSLOTS, 30000)
    i1s = rng.integers(0, NCORES, 30000)
    i2s = rng.integers(0, NCORES, 30000)
    for s1, s2, i1, i2 in zip(s1s, s2s, i1s, i2s):
        if s1 == s2:
            continue
        g1 = groups[s1][:]
        g2 = groups[s2][:]
        g1[i1], g2[i2] = groups[s2][i2], groups[s1][i1]
        n1, n2 = gcost(g1), gcost(g2)
        if n1 + n2 < cost[s1] + cost[s2] - 1e-9:
            groups[s1], groups[s2] = g1, g2
            cost[s1], cost[s2] = n1, n2
    slot_at = [max(at[i] for i in g) for g in groups]
    slot_bt = [max(bt[i] for i in g) for g in groups]
    # run small slots first: minimizes the pipeline-fill bubble
    sorder = sorted(range(NSLOTS), key=lambda s: cost[s])
    groups = [groups[s] for s in sorder]
    slot_at = [slot_at[s] for s in sorder]
    slot_bt = [slot_bt[s] for s in sorder]
    return swap, qa, qb, groups, slot_at, slot_bt


def _build_program(slot_at, slot_bt):
    import concourse.bass as bass  # noqa: F401
    import concourse.mybir as mybir
    import concourse.tile as tile
    from concourse.tile import add_dep_helper
    from concourse import bacc

    F32 = mybir.dt.float32
    F32R = mybir.dt.float32r
    BF16 = mybir.dt.bfloat16
    FP8 = mybir.dt.float8e4
    DR = mybir.MatmulPerfMode.DoubleRow
    Exp = mybir.ActivationFunctionType.Exp
    Ident = mybir.ActivationFunctionType.Identity

    tot_at = sum(slot_at)
    tot_bt = sum(slot_bt)
    cum_at = np.concatenate([[0], np.cumsum(slot_at)]).astype(int)
    cum_bt = np.concatenate([[0], np.cumsum(slot_bt)]).astype(int)

    nc = bacc.Bacc("TRN2", target_bir_lowering=False, debug=False,
                   num_devices=NCORES)

    tot = tot_at + tot_bt
    # natural layout (row-tiled; per slot A-rows then B-rows) for u = w^T B
    nat_d = nc.dram_tensor("nat", [tot * P, DIM], BF16, kind="ExternalInput")
    # host-transposed layout [dpart, dt, seq] (per slot A-cols then B-cols)
    tr_d = nc.dram_tensor("tr", [P, DT, tot * P], BF16, kind="ExternalInput")
    # all small per-core constants packed into one tensor:
    # [gs_a | gs_b | npa | npb | bqs | bk] along the free dim
    NSM = tot_at + tot_bt + NSLOTS + NSLOTS + MI + MI
    sm_d = nc.dram_tensor("sm", [P, NSM], F32, kind="ExternalInput")
    wq_d = nc.dram_tensor("wq", [P, DT, INNER], BF16, kind="ExternalInput")
    wk_d = nc.dram_tensor("wk", [P, DT, INNER], BF16, kind="ExternalInput")
    wv_d = nc.dram_tensor("wv", [P, DT, OUTER], BF16, kind="ExternalInput")
    bvb_d = nc.dram_tensor("bvb", [2 * NSLOTS, OUTER], F32,
                           kind="ExternalInput")
    idr_d = nc.dram_tensor("idr", [P, P], F32R, kind="ExternalInput")
    emb_d = nc.dram_tensor("emb", [2 * NSLOTS, OUTER], F32,
                           kind="ExternalOutput")
    # DRAM bounce buffer for the w row->col partition scatter
    wsc_d = nc.dram_tensor("wsc", [2 * NSLOTS, NCORES * P], BF16,
                           kind="ExternalOutput")
    cum = np.concatenate([[0], np.cumsum(
        [slot_at[s] + slot_bt[s] for s in range(NSLOTS)])]).astype(int)

    with tile.TileContext(nc) as tc:
        with (
            tc.tile_pool(name="const", bufs=1) as cpool,
            tc.tile_pool(name="anat", bufs=2) as apool,
            tc.tile_pool(name="atr", bufs=2) as atpool,
            tc.tile_pool(name="proj", bufs=2) as ppool,
            tc.tile_pool(name="pexp", bufs=2) as epool,
            tc.tile_pool(name="small", bufs=3) as spool,
            tc.tile_pool(name="late", bufs=2) as lpool,
            tc.tile_pool(name="psB", bufs=3, space="PSUM") as psB,
            tc.tile_pool(name="psW", bufs=1, space="PSUM") as psW,
        ):
            # ---- constants ----
            wq_sb = cpool.tile([P, DT, INNER], BF16, tag="wq")
            wk_sb = cpool.tile([P, DT, INNER], BF16, tag="wk")
            wv_sb = cpool.tile([P, DT, OUTER], BF16, tag="wv")
            bvb_sb = cpool.tile([2 * NSLOTS, OUTER], F32, tag="bvb")
            idr_sb = cpool.tile([P, P], F32R, tag="idr")
            sm_sb = cpool.tile([P, NSM], F32, tag="sm")
            # column offsets into sm_sb: [gs_a | gs_b | npa | npb | bqs | bk]
            GA, GB = 0, tot_at
            NPA, NPB = tot_at + tot_bt, tot_at + tot_bt + NSLOTS
            BQ, BK = NSM - 2 * MI, NSM - MI
            urows_sb = cpool.tile([2 * NSLOTS, DIM], F32R, tag="urows")
            idb2_sb = cpool.tile([1, 2], BF16, tag="idb2")
            nc.vector.memset(idb2_sb[0:1, 0:1], 1.0)
            nc.vector.memset(idb2_sb[0:1, 1:2], 0.0)

            ev = 0  # evac engine alternator
            for s in range(NSLOTS):
                at_s, bt_s = int(slot_at[s]), int(slot_bt[s])
                pla, plb = at_s * P, bt_s * P
                nt = at_s + bt_s
                # ---- loads: transposed first (projections), natural later ----
                tr_sb = atpool.tile([P, DT, nt * P], BF16, tag="tr")
                nc.sync.dma_start(
                    tr_sb[:], tr_d[:, :, cum[s] * P:cum[s + 1] * P])
                if s == 0:
                    # slot-0 inputs are already in flight; now the rest
                    nc.sync.dma_start(sm_sb[:], sm_d[:])
                    nc.sync.dma_start(wq_sb[:], wq_d[:])
                    nc.sync.dma_start(wk_sb[:], wk_d[:])
                nat_sb = apool.tile([P, nt, DIM], BF16, tag="nat")
                nc.sync.dma_start(
                    nat_sb[:], nat_d[cum[s] * P:cum[s + 1] * P, :]
                    .rearrange("(t p) d -> p t d", p=P))

                # ---- projections (order: dir-A deps first) ----
                # q/k stored fp8-e4m3: scores then run one DoubleRow matmul
                # per chunk (0.5 cyc/row, all 256 contraction at once)
                qaT = ppool.tile([P, MI, pla], FP8, tag="qaT")
                kaT = ppool.tile([P, MI, pla], FP8, tag="kaT")
                qbT = ppool.tile([P, MI, plb], FP8, tag="qbT")
                kbT = ppool.tile([P, MI, plb], FP8, tag="kbT")
                for dst, soff, pl, w_sb, bo in (
                        (qaT, 0, pla, wq_sb, BQ),
                        (kbT, pla, plb, wk_sb, BK),
                        (kaT, 0, pla, wk_sb, BK),
                        (qbT, pla, plb, wq_sb, BQ)):
                    for m in range(MI):
                        pp = psB.tile([P, 1024], F32, tag="big")
                        for kt in range(DT):
                            for noff, nlen in _chunks(pl):
                                nc.tensor.matmul(
                                    pp[:, noff:noff + nlen],
                                    w_sb[:, kt, m * P:(m + 1) * P],
                                    tr_sb[:, kt,
                                          soff + noff:soff + noff + nlen],
                                    start=(kt == 0), stop=(kt == DT - 1))
                        if ev % 2 == 0:
                            nc.vector.tensor_scalar_add(
                                dst[:, m, :], pp[:, :pl],
                                sm_sb[:, bo + m, None])
                        else:
                            nc.scalar.activation(
                                dst[:, m, :], pp[:, :pl],
                                Ident, bias=sm_sb[:, bo + m, None], scale=1.0)
                        ev += 1

                # ---- attention directions ----
                for dr in range(2):
                    if dr == 0:  # A queries over B keys
                        QT, KT, nq, nk = qaT, kbT, at_s, bt_s
                        g_off = GA + cum_at[s]
                        np_off = NPB + s
                        koff = at_s  # B rows sit after A rows in nat_sb
                    else:
                        QT, KT, nq, nk = qbT, kaT, bt_s, at_s
                        g_off = GB + cum_bt[s]
                        np_off = NPA + s
                        koff = 0
                    plk = nk * P
                    kchunks = _chunks(plk)
                    wr = [psW.tile([1, cl], F32, tag=f"wr{ci}",
                                   name=f"wr{ci}")
                          for ci, (co, cl) in enumerate(kchunks)]
                    for qt in range(nq):
                        sc = psB.tile([P, 1024], F32, tag="big")
                        for co, cl in kchunks:
                            nc.tensor.matmul(
                                sc[:, co:co + cl],
                                QT[:, :, qt * P:(qt + 1) * P],
                                KT[:, :, co:co + cl],
                                start=True, stop=True, perf_mode=DR)
                        # pad contribution npad*p_pad as exp(s_pad+ln(npad))
                        # in f32 (bf16 p_pad would amplify through the
                        # den - npad*p_pad cancellation); np_sb holds ln(npad).
                        # Emitted BEFORE the big exp so the vector chain can
                        # overlap it.
                        pc = spool.tile([P, 1], F32, tag="pc")
                        nc.scalar.activation(
                            pc[:], sc[:, plk - 1:plk], Exp,
                            bias=sm_sb[:, np_off, None], scale=SCALE)
                        den = spool.tile([P, 1], F32, tag="den")
                        p_sb = epool.tile([P, plk], BF16, tag="p_sb")
                        nc.scalar.activation(
                            p_sb[:], sc[:, :plk], Exp,
                            bias=0.0, scale=SCALE, accum_out=den[:])
                        # den_f = pad - den  (= -true_den; g is negated on
                        # host so gcol comes out positive)
                        denf = spool.tile([P, 1], F32, tag="denf")
                        nc.vector.tensor_sub(denf[:], pc[:], den[:])
                        rec = spool.tile([P, 1], F32, tag="rec")
                        nc.vector.reciprocal(rec[:], denf[:])
                        gcol = spool.tile([P, 1], BF16, tag="gcol")
                        nc.vector.tensor_mul(gcol[:], rec[:],
                                             sm_sb[:, g_off + qt, None])
                        for ci, (co, cl) in enumerate(kchunks):
                            nc.tensor.matmul(
                                wr[ci][:], gcol[:], p_sb[:, co:co + cl],
                                start=(qt == 0), stop=(qt == nq - 1))
                    # w row -> w col (transpose via identity matmuls; the
                    # 2-wide output keeps PSUM writes 8B-aligned)
                    wrow = lpool.tile([1, plk], BF16, tag="wrow")
                    for ci, (co, cl) in enumerate(kchunks):
                        nc.scalar.copy(wrow[0:1, co:co + cl], wr[ci][:])
                    wt = psB.tile([P, 1024], F32, tag="big")
                    for kt in range(nk):
                        nc.tensor.matmul(
                            wt[:, 2 * kt:2 * kt + 2],
                            wrow[0:1, kt * P:(kt + 1) * P],
                            idb2_sb[0:1, 0:2], start=True, stop=True)
                    wcol = lpool.tile([P, nk], BF16, tag="wcol")
                    nc.vector.tensor_copy(
                        wcol[:],
                        wt[:, :2 * nk].rearrange(
                            "p (k two) -> p k two", two=2)[:, :, 0])
                    # u row = w^T @ Knat
                    ur = psB.tile([P, 1024], F32, tag="big")
                    for noff, nlen in _chunks(DIM):
                        for kt in range(nk):
                            nc.tensor.matmul(
                                ur[0:1, noff:noff + nlen],
                                wcol[:, kt:kt + 1],
                                nat_sb[:, koff + kt, noff:noff + nlen],
                                start=(kt == 0), stop=(kt == nk - 1))
                    ursb = lpool.tile([1, DIM], F32R, tag="ursb")
                    nc.scalar.copy(ursb[:], ur[0:1, :DIM])
                    nc.sync.dma_start(
                        urows_sb[2 * s + dr:2 * s + dr + 1, :], ursb[:])

            # ---- final: E^T = U^T Wv + bv ----
            for sb, d in ((wv_sb, wv_d), (bvb_sb, bvb_d), (idr_sb, idr_d)):
                nc.sync.dma_start(sb[:], d[:])
            u_sb = cpool.tile([P, DT, 2 * NSLOTS], BF16, tag="usb")
            for dt in range(DT):
                ut = psB.tile([P, 1024], F32, tag="big")
                nc.tensor.matmul(
                    ut[:, :2 * NSLOTS],
                    urows_sb[:, dt * P:(dt + 1) * P],
                    idr_sb[0:2 * NSLOTS, 0:2 * NSLOTS],
                    start=True, stop=True)
                nc.vector.tensor_copy(u_sb[:, dt, :], ut[:, :2 * NSLOTS])
            eT = psB.tile([P, 1024], F32, tag="big")
            for noff, nlen in _chunks(OUTER):
                for dt in range(DT):
                    nc.tensor.matmul(
                        eT[0:2 * NSLOTS, noff:noff + nlen],
                        u_sb[:, dt, :],
                        wv_sb[:, dt, noff:noff + nlen],
                        start=(dt == 0), stop=(dt == DT - 1))
            e_sb = cpool.tile([2 * NSLOTS, OUTER], F32, tag="esb")
            nc.vector.tensor_add(e_sb[:], eT[0:2 * NSLOTS, :], bvb_sb[:])
            nc.sync.dma_start(emb_d[:], e_sb[:])

    nc.compile()
    return nc


def _install_profhook():
    import contextlib
    import ctypes
    import types

    import antenv

    if not hasattr(antenv, "axon_hooks"):
        mod = types.ModuleType("antenv.axon_hooks")
        mod._hook = None

        def _set(h):
            mod._hook = h

        def _get():
            return mod._hook

        mod.set_axon_ntff_profile_hook = _set
        mod.get_axon_ntff_profile_hook = _get
        sys.modules["antenv.axon_hooks"] = mod
        antenv.axon_hooks = mod
    from antenv.axon_hooks import set_axon_ntff_profile_hook
    so_path = "/opt/axon/libaxon_pjrt.so"
    if not os.path.exists(so_path):
        return False
    lib = ctypes.CDLL(so_path)
    if not hasattr(lib, "axon_start_nrt_profile"):
        return False
    lib.axon_start_nrt_profile.argtypes = [ctypes.POINTER(ctypes.c_int64),
                                           ctypes.c_size_t]
    lib.axon_start_nrt_profile.restype = ctypes.c_int64
    lib.axon_stop_nrt_profile.argtypes = [ctypes.c_char_p]
    lib.axon_stop_nrt_profile.restype = ctypes.c_int64

    @contextlib.contextmanager
    def _hook(output_dir, device_ids):
        import jax

        jax.devices()
        if device_ids:
            ids = (ctypes.c_int64 * len(device_ids))(*device_ids)
            rc = lib.axon_start_nrt_profile(ids, len(device_ids))
        else:
            rc = lib.axon_start_nrt_profile(None, 0)
        if rc != 0:
            raise RuntimeError(f"axon_start_nrt_profile rc={rc}")
        try:
            yield
        finally:
            n = lib.axon_stop_nrt_profile(str(output_dir).encode())
            print(f"profile: {n} file(s) written to {output_dir}",
                  file=sys.stderr)

    set_axon_ntff_profile_hook(_hook)
    return True


def kernel(a_pad, b_pad, len_a, len_b, Wq, bq, Wk, bk, Wv, bv):
    global LAST_EXEC_TIME_NS
    import ml_dtypes
    BF = ml_dtypes.bfloat16

    a_pad = np.ascontiguousarray(np.asarray(a_pad, np.float32))
    b_pad = np.ascontiguousarray(np.asarray(b_pad, np.float32))
    len_a = np.asarray(len_a, np.int32)
    len_b = np.asarray(len_b, np.int32)
    Wq = np.asarray(Wq, np.float32)
    Wk = np.asarray(Wk, np.float32)
    Wv = np.asarray(Wv, np.float32)
    bq = np.asarray(bq, np.float32)
    bk = np.asarray(bk, np.float32)
    bv = np.asarray(bv, np.float32)

    swap, qa_len, qb_len, groups, slot_at, slot_bt = _plan(len_a, len_b)
    tot_at, tot_bt = sum(slot_at), sum(slot_bt)
    cum_at = np.concatenate([[0], np.cumsum(slot_at)]).astype(int)
    cum_bt = np.concatenate([[0], np.cumsum(slot_bt)]).astype(int)
    tot = tot_at + tot_bt
    cum = np.concatenate([[0], np.cumsum(
        [slot_at[s] + slot_bt[s] for s in range(NSLOTS)])]).astype(int)
    NSM = tot_at + tot_bt + 2 * NSLOTS + 2 * MI

    # ---- shared (per-core-identical) inputs ----
    # 1/sqrt(INNER) applied via the exp activation's scale argument, so q
    # stays ~N(0,1) for fp8 storage
    wq_h = Wq.reshape(DT, P, INNER).transpose(1, 0, 2).astype(BF)
    wk_h = Wk.reshape(DT, P, INNER).transpose(1, 0, 2).astype(BF)
    wv_h = Wv.reshape(DT, P, OUTER).transpose(1, 0, 2).astype(BF)
    bqs_h = bq.reshape(MI, P).T.copy()
    bk_h = bk.reshape(MI, P).T.copy()
    bvb_h = np.broadcast_to(bv, (2 * NSLOTS, OUTER)).copy()
    idr_h = np.eye(P, dtype=np.float32)

    a16 = a_pad.astype(BF)
    b16 = b_pad.astype(BF)

    # ---- per-core inputs ----
    in_maps = []
    for c in range(NCORES):
        nat = np.zeros((tot * P, DIM), BF)
        tr = np.zeros((P, DT, tot * P), BF)
        sm = np.zeros((P, NSM), np.float32)
        gs_a = sm[:, 0:tot_at]
        gs_b = sm[:, tot_at:tot_at + tot_bt]
        npa = sm[:, tot_at + tot_bt:tot_at + tot_bt + NSLOTS]
        npb = sm[:, tot_at + tot_bt + NSLOTS:tot_at + tot_bt + 2 * NSLOTS]
        sm[:, NSM - 2 * MI:NSM - MI] = bqs_h
        sm[:, NSM - MI:NSM] = bk_h
        for s in range(NSLOTS):
            i = groups[s][c]
            la_i, lb_i = int(qa_len[i]), int(qb_len[i])
            A = b16[i] if swap[i] else a16[i]
            Bm = a16[i] if swap[i] else b16[i]
            ao = cum[s] * P                  # A rows/cols at slot start
            bo = (cum[s] + slot_at[s]) * P   # B rows/cols after A's
            nat[ao:ao + la_i] = A[:la_i]
            nat[bo:bo + lb_i] = Bm[:lb_i]
            # transposed layout: [dpart, dt, seq]
            tr[:, :, ao:ao + la_i] = \
                A[:la_i].T.reshape(DT, P, la_i).transpose(1, 0, 2)
            tr[:, :, bo:bo + lb_i] = \
                Bm[:lb_i].T.reshape(DT, P, lb_i).transpose(1, 0, 2)
            # g columns NEGATED (sign trick pairs with den_f = pc - den)
            ga = np.zeros(slot_at[s] * P, np.float32)
            ga[:la_i] = -1.0 / la_i
            gs_a[:, cum_at[s]:cum_at[s] + slot_at[s]] = \
                ga.reshape(slot_at[s], P).T
            gb = np.zeros(slot_bt[s] * P, np.float32)
            gb[:lb_i] = -1.0 / lb_i
            gs_b[:, cum_bt[s]:cum_bt[s] + slot_bt[s]] = \
                gb.reshape(slot_bt[s], P).T
            na_i = slot_at[s] * P - la_i
            nb_i = slot_bt[s] * P - lb_i
            npa[:, s] = np.log(na_i) if na_i > 0 else -1e30
            npb[:, s] = np.log(nb_i) if nb_i > 0 else -1e30
        in_maps.append({
            "nat": nat, "tr": tr, "sm": sm,
            "wq": wq_h, "wk": wk_h, "wv": wv_h,
            "bvb": bvb_h, "idr": idr_h,
        })

    nc = _build_program(slot_at, slot_bt)

    from concourse.bass_utils import run_bass_kernel_spmd

    trace = os.environ.get("BASS_KERNEL_TRACE", "0") == "1"
    if trace:
        _install_profhook()
    res = run_bass_kernel_spmd(nc, in_maps, list(range(NCORES)), trace=trace)
    LAST_EXEC_TIME_NS = res.exec_time_ns

    emb_a = np.zeros((B, OUTER), np.float32)
    emb_b = np.zeros((B, OUTER), np.float32)
    for c in range(NCORES):
        e = np.asarray(res.results[c]["emb"], np.float32)
        for s in range(NSLOTS):
            i = groups[s][c]
            ea, eb = e[2 * s], e[2 * s + 1]  # A-queries, B-queries
            if swap[i]:
                emb_a[i], emb_b[i] = eb, ea
            else:
                emb_a[i], emb_b[i] = ea, eb
    return emb_a, emb_b


# revision 45
# speedup vs baseline: 1.4481x; 1.1201x over previous
"""Ragged cross-attention pooling kernel for Trainium2 (8 NeuronCores, SPMD).

Math (per pair, direction "A attends over B"):
    qa = (A @ Wq*scale + bq*scale)      [la, INNER]
    kb =  B @ Wk + bk                   [lb, INNER]
    s  = qa @ kb^T                      [la, lb]
    p  = exp(s)               (no max-subtraction needed: |s| <~ 6)
    den[q] = sum_k p[q, k]  (pad-corrected: all pad cols share p[:, -1])
    gcol[q] = valid(q) / (la * den[q])
    w[k] = sum_q gcol[q] p[q, k]        <- collapses the mean over queries
    emb  = (w^T B) @ Wv + bv            <- collapses attn@V and the V projection

v2: A/B pre-transposed ON HOST (no on-chip transposes), all matmul inputs
bf16 (1 cyc/row at any moving size), single wide exp per query tile into a
2-bank PSUM tile, w row->col via SBUF-to-SBUF scatter DMA, final E computed
as E^T = U^T Wv with 16-wide stationary.

Distribution: 64 pairs -> 8 slots x 8 cores (one shared SPMD program, shapes
fixed per slot to the max over cores; pairs bin-packed by length so padding is
small).
"""

import os
import sys

sys.path.insert(0, "/opt/trn_rl_repo")

import numpy as np

B, LA, LB, DIM, INNER, OUTER = 64, 1024, 1024, 640, 256, 1024
NCORES, NSLOTS, P = 8, 8, 128
SCALE = 1.0 / np.sqrt(INNER)
DT = DIM // P  # 5 d-chunks
MI = INNER // P  # 2 inner-chunks

LAST_EXEC_TIME_NS = None


def _chunks(total, cap=512):
    out, off = [], 0
    while off < total:
        c = min(cap, total - off)
        out.append((off, c))
        off += c
    return out


def _plan(la_all, lb_all):
    """Assign pairs to (slot, core); returns swap flags, groups, slot tile shapes."""
    la = np.asarray(la_all, np.int64)
    lb = np.asarray(lb_all, np.int64)
    swap = lb > la
    qa = np.where(swap, lb, la)  # kernel A-side length (>= B-side)
    qb = np.where(swap, la, lb)
    at = -(-qa // P)
    bt = -(-qb // P)
    order = np.argsort(-(at * 1024 + bt), kind="stable")
    groups = [list(order[s * NCORES:(s + 1) * NCORES]) for s in range(NSLOTS)]
    C1, C2 = 1000.0, 450.0

    def gcost(g):
        ma = max(at[i] for i in g)
        mb = max(bt[i] for i in g)
        return C1 * (ma + mb) + C2 * ma * mb

    rng = np.random.default_rng(0)
    cost = [gcost(g) for g in groups]
    NIT = 120000
    s1s = rng.integers(0, NSLOTS, NIT)
    s2s = rng.integers(0, NSLOTS, NIT)
    i1s = rng.integers(0, NCORES, NIT)
    i2s = rng.integers(0, NCORES, NIT)
    for s1, s2, i1, i2 in zip(s1s, s2s, i1s, i2s):
        if s1 == s2:
            continue
        g1 = groups[s1][:]
        g2 = groups[s2][:]
        g1[i1], g2[i2] = groups[s2][i2], groups[s1][i1]
        n1, n2 = gcost(g1), gcost(g2)
        if n1 + n2 < cost[s1] + cost[s2] - 1e-9:
            groups[s1], groups[s2] = g1, g2
            cost[s1], cost[s2] = n1, n2
    slot_at = [max(at[i] for i in g) for g in groups]
    slot_bt = [max(bt[i] for i in g) for g in groups]
    # run small slots first: minimizes the pipeline-fill bubble
    sorder = sorted(range(NSLOTS), key=lambda s: cost[s])
    groups = [groups[s] for s in sorder]
    slot_at = [slot_at[s] for s in sorder]
    slot_bt = [slot_bt[s] for s in sorder]
    return swap, qa, qb, groups, slot_at, slot_bt


def _build_program(slot_at, slot_bt):
    import concourse.bass as bass  # noqa: F401
    import concourse.mybir as mybir
    import concourse.tile as tile
    from concourse.tile import add_dep_helper
    from concourse import bacc

    F32 = mybir.dt.float32
    F32R = mybir.dt.float32r
    BF16 = mybir.dt.bfloat16
    FP8 = mybir.dt.float8e4
    DR = mybir.MatmulPerfMode.DoubleRow
    Exp = mybir.ActivationFunctionType.Exp
    Ident = mybir.ActivationFunctionType.Identity

    tot_at = sum(slot_at)
    tot_bt = sum(slot_bt)
    cum_at = np.concatenate([[0], np.cumsum(slot_at)]).astype(int)
    cum_bt = np.concatenate([[0], np.cumsum(slot_bt)]).astype(int)

    nc = bacc.Bacc("TRN2", target_bir_lowering=False, debug=False,
                   num_devices=NCORES)

    tot = tot_at + tot_bt
    # natural layout (row-tiled; per slot A-rows then B-rows) for u = w^T B
    nat_d = nc.dram_tensor("nat", [tot * P, DIM], BF16, kind="ExternalInput")
    # host-transposed layout [dpart, dt, seq] (per slot A-cols then B-cols);
    # fp8 so projections run DoubleRow
    tr_d = nc.dram_tensor("tr", [P, DT, tot * P], FP8, kind="ExternalInput")
    # all small per-core constants packed into one tensor:
    # [gs_a | gs_b | npa | npb | bqs | bk] along the free dim
    NSM = tot_at + tot_bt + NSLOTS + NSLOTS + MI + MI
    sm_d = nc.dram_tensor("sm", [P, NSM], F32, kind="ExternalInput")
    wq_d = nc.dram_tensor("wq", [P, DT, INNER], FP8, kind="ExternalInput")
    wk_d = nc.dram_tensor("wk", [P, DT, INNER], FP8, kind="ExternalInput")
    wv_d = nc.dram_tensor("wv", [P, DT, OUTER], BF16, kind="ExternalInput")
    bvb_d = nc.dram_tensor("bvb", [2 * NSLOTS, OUTER], F32,
                           kind="ExternalInput")
    idr_d = nc.dram_tensor("idr", [P, P], F32R, kind="ExternalInput")
    emb_d = nc.dram_tensor("emb", [2 * NSLOTS, OUTER], F32,
                           kind="ExternalOutput")
    # DRAM bounce buffer for the w row->col partition scatter
    wsc_d = nc.dram_tensor("wsc", [2 * NSLOTS, NCORES * P], BF16,
                           kind="ExternalOutput")
    cum = np.concatenate([[0], np.cumsum(
        [slot_at[s] + slot_bt[s] for s in range(NSLOTS)])]).astype(int)

    with tile.TileContext(nc) as tc:
        with (
            tc.tile_pool(name="const", bufs=1) as cpool,
            tc.tile_pool(name="anat", bufs=2) as apool,
            tc.tile_pool(name="atr", bufs=2) as atpool,
            tc.tile_pool(name="proj", bufs=2) as ppool,
            tc.tile_pool(name="pexp", bufs=2) as epool,
            tc.tile_pool(name="small", bufs=3) as spool,
            tc.tile_pool(name="late", bufs=2) as lpool,
            tc.tile_pool(name="psB", bufs=3, space="PSUM") as psB,
            tc.tile_pool(name="psW", bufs=1, space="PSUM") as psW,
        ):
            # ---- constants ----
            wq_sb = cpool.tile([P, DT, INNER], FP8, tag="wq")
            wk_sb = cpool.tile([P, DT, INNER], FP8, tag="wk")
            wv_sb = cpool.tile([P, DT, OUTER], BF16, tag="wv")
            bvb_sb = cpool.tile([2 * NSLOTS, OUTER], F32, tag="bvb")
            idr_sb = cpool.tile([P, P], F32R, tag="idr")
            sm_sb = cpool.tile([P, NSM], F32, tag="sm")
            # column offsets into sm_sb: [gs_a | gs_b | npa | npb | bqs | bk]
            GA, GB = 0, tot_at
            NPA, NPB = tot_at + tot_bt, tot_at + tot_bt + NSLOTS
            BQ, BK = NSM - 2 * MI, NSM - MI
            urows_sb = cpool.tile([2 * NSLOTS, DIM], F32R, tag="urows")
            idb2_sb = cpool.tile([1, 2], BF16, tag="idb2")
            nc.vector.memset(idb2_sb[0:1, 0:1], 1.0)
            nc.vector.memset(idb2_sb[0:1, 1:2], 0.0)

            ev = 0  # evac engine alternator
            for s in range(NSLOTS):
                at_s, bt_s = int(slot_at[s]), int(slot_bt[s])
                pla, plb = at_s * P, bt_s * P
                nt = at_s + bt_s
                # ---- loads: transposed first (projections), natural later ----
                tr_sb = atpool.tile([P, DT, nt * P], FP8, tag="tr")
                nc.sync.dma_start(
                    tr_sb[:], tr_d[:, :, cum[s] * P:cum[s + 1] * P])
                if s == 0:
                    # slot-0 inputs are already in flight; now the rest
                    nc.sync.dma_start(sm_sb[:], sm_d[:])
                    nc.sync.dma_start(wq_sb[:], wq_d[:])
                    nc.sync.dma_start(wk_sb[:], wk_d[:])
                nat_sb = apool.tile([P, nt, DIM], BF16, tag="nat")
                nc.sync.dma_start(
                    nat_sb[:], nat_d[cum[s] * P:cum[s + 1] * P, :]
                    .rearrange("(t p) d -> p t d", p=P))

                # ---- projections (order: dir-A deps first) ----
                # q/k stored fp8-e4m3: scores then run one DoubleRow matmul
                # per chunk (0.5 cyc/row, all 256 contraction at once)
                qaT = ppool.tile([P, MI, pla], FP8, tag="qaT")
                kaT = ppool.tile([P, MI, pla], FP8, tag="kaT")
                qbT = ppool.tile([P, MI, plb], FP8, tag="qbT")
                kbT = ppool.tile([P, MI, plb], FP8, tag="kbT")
                for dst, soff, pl, w_sb, bo in (
                        (qaT, 0, pla, wq_sb, BQ),
                        (kbT, pla, plb, wk_sb, BK),
                        (kaT, 0, pla, wk_sb, BK),
                        (qbT, pla, plb, wq_sb, BQ)):
                    for m in range(MI):
                        pp = psB.tile([P, 1024], F32, tag="big")
                        for noff, nlen in _chunks(pl):
                            # DT=5 contraction tiles: 2 DoubleRow pair
                            # matmuls + 1 plain fp8 matmul
                            for g in range(2):
                                nc.tensor.matmul(
                                    pp[:, noff:noff + nlen],
                                    w_sb[:, 2 * g:2 * g + 2,
                                         m * P:(m + 1) * P],
                                    tr_sb[:, 2 * g:2 * g + 2,
                                          soff + noff:soff + noff + nlen],
                                    start=(g == 0), stop=False,
                                    perf_mode=DR)
                            nc.tensor.matmul(
                                pp[:, noff:noff + nlen],
                                w_sb[:, DT - 1, m * P:(m + 1) * P],
                                tr_sb[:, DT - 1,
                                      soff + noff:soff + noff + nlen],
                                start=False, stop=True)
                        if ev % 2 == 0:
                            nc.vector.tensor_scalar_add(
                                dst[:, m, :], pp[:, :pl],
                                sm_sb[:, bo + m, None])
                        else:
                            nc.scalar.activation(
                                dst[:, m, :], pp[:, :pl],
                                Ident, bias=sm_sb[:, bo + m, None], scale=1.0)
                        ev += 1

                # ---- attention directions ----
                for dr in range(2):
                    if dr == 0:  # A queries over B keys
                        QT, KT, nq, nk = qaT, kbT, at_s, bt_s
                        g_off = GA + cum_at[s]
                        np_off = NPB + s
                        koff = at_s  # B rows sit after A rows in nat_sb
                    else:
                        QT, KT, nq, nk = qbT, kaT, bt_s, at_s
                        g_off = GB + cum_bt[s]
                        np_off = NPA + s
                        koff = 0
                    plk = nk * P
                    kchunks = _chunks(plk)
                    wr = [psW.tile([1, cl], F32, tag=f"wr{ci}",
                                   name=f"wr{ci}")
                          for ci, (co, cl) in enumerate(kchunks)]
                    for qt in range(nq):
                        sc = psB.tile([P, 1024], F32, tag="big")
                        for co, cl in kchunks:
                            nc.tensor.matmul(
                                sc[:, co:co + cl],
                                QT[:, :, qt * P:(qt + 1) * P],
                                KT[:, :, co:co + cl],
                                start=True, stop=True, perf_mode=DR)
                        # pad contribution npad*p_pad as exp(s_pad+ln(npad))
                        # in f32 (bf16 p_pad would amplify through the
                        # den - npad*p_pad cancellation); np_sb holds ln(npad).
                        # Emitted BEFORE the big exp so the vector chain can
                        # overlap it.
                        pc = spool.tile([P, 1], F32, tag="pc")
                        nc.scalar.activation(
                            pc[:], sc[:, plk - 1:plk], Exp,
                            bias=sm_sb[:, np_off, None], scale=SCALE)
                        den = spool.tile([P, 1], F32, tag="den")
                        p_sb = epool.tile([P, plk], BF16, tag="p_sb")
                        nc.scalar.activation(
                            p_sb[:], sc[:, :plk], Exp,
                            bias=0.0, scale=SCALE, accum_out=den[:])
                        # den_f = pad - den  (= -true_den; g is negated on
                        # host so gcol comes out positive)
                        denf = spool.tile([P, 1], F32, tag="denf")
                        nc.vector.tensor_sub(denf[:], pc[:], den[:])
                        rec = spool.tile([P, 1], F32, tag="rec")
                        nc.vector.reciprocal(rec[:], denf[:])
                        gcol = spool.tile([P, 1], BF16, tag="gcol")
                        nc.vector.tensor_mul(gcol[:], rec[:],
                                             sm_sb[:, g_off + qt, None])
                        for ci, (co, cl) in enumerate(kchunks):
                            nc.tensor.matmul(
                                wr[ci][:], gcol[:], p_sb[:, co:co + cl],
                                start=(qt == 0), stop=(qt == nq - 1))
                    # w row -> w col (transpose via identity matmuls; the
                    # 2-wide output keeps PSUM writes 8B-aligned)
                    wrow = lpool.tile([1, plk], BF16, tag="wrow")
                    for ci, (co, cl) in enumerate(kchunks):
                        nc.scalar.copy(wrow[0:1, co:co + cl], wr[ci][:])
                    wt = psB.tile([P, 1024], F32, tag="big")
                    for kt in range(nk):
                        nc.tensor.matmul(
                            wt[:, 2 * kt:2 * kt + 2],
                            wrow[0:1, kt * P:(kt + 1) * P],
                            idb2_sb[0:1, 0:2], start=True, stop=True)
                    wcol = lpool.tile([P, nk], BF16, tag="wcol")
                    nc.vector.tensor_copy(
                        wcol[:],
                        wt[:, :2 * nk].rearrange(
                            "p (k two) -> p k two", two=2)[:, :, 0])
                    # u row = w^T @ Knat
                    ur = psB.tile([P, 1024], F32, tag="big")
                    for noff, nlen in _chunks(DIM):
                        for kt in range(nk):
                            nc.tensor.matmul(
                                ur[0:1, noff:noff + nlen],
                                wcol[:, kt:kt + 1],
                                nat_sb[:, koff + kt, noff:noff + nlen],
                                start=(kt == 0), stop=(kt == nk - 1))
                    ursb = lpool.tile([1, DIM], F32R, tag="ursb")
                    nc.scalar.copy(ursb[:], ur[0:1, :DIM])
                    nc.sync.dma_start(
                        urows_sb[2 * s + dr:2 * s + dr + 1, :], ursb[:])

            # ---- final: E^T = U^T Wv + bv ----
            for sb, d in ((wv_sb, wv_d), (bvb_sb, bvb_d), (idr_sb, idr_d)):
                nc.sync.dma_start(sb[:], d[:])
            u_sb = cpool.tile([P, DT, 2 * NSLOTS], BF16, tag="usb")
            for dt in range(DT):
                ut = psB.tile([P, 1024], F32, tag="big")
                nc.tensor.matmul(
                    ut[:, :2 * NSLOTS],
                    urows_sb[:, dt * P:(dt + 1) * P],
                    idr_sb[0:2 * NSLOTS, 0:2 * NSLOTS],
                    start=True, stop=True)
                nc.vector.tensor_copy(u_sb[:, dt, :], ut[:, :2 * NSLOTS])
            eT = psB.tile([P, 1024], F32, tag="big")
            for noff, nlen in _chunks(OUTER):
                for dt in range(DT):
                    nc.tensor.matmul(
                        eT[0:2 * NSLOTS, noff:noff + nlen],
                        u_sb[:, dt, :],
                        wv_sb[:, dt, noff:noff + nlen],
                        start=(dt == 0), stop=(dt == DT - 1))
            e_sb = cpool.tile([2 * NSLOTS, OUTER], F32, tag="esb")
            nc.vector.tensor_add(e_sb[:], eT[0:2 * NSLOTS, :], bvb_sb[:])
            nc.sync.dma_start(emb_d[:], e_sb[:])

    nc.compile()
    return nc


def _install_profhook():
    import contextlib
    import ctypes
    import types

    import antenv

    if not hasattr(antenv, "axon_hooks"):
        mod = types.ModuleType("antenv.axon_hooks")
        mod._hook = None

        def _set(h):
            mod._hook = h

        def _get():
            return mod._hook

        mod.set_axon_ntff_profile_hook = _set
        mod.get_axon_ntff_profile_hook = _get
        sys.modules["antenv.axon_hooks"] = mod
        antenv.axon_hooks = mod
    from antenv.axon_hooks import set_axon_ntff_profile_hook
    so_path = "/opt/axon/libaxon_pjrt.so"
    if not os.path.exists(so_path):
        return False
    lib = ctypes.CDLL(so_path)
    if not hasattr(lib, "axon_start_nrt_profile"):
        return False
    lib.axon_start_nrt_profile.argtypes = [ctypes.POINTER(ctypes.c_int64),
                                           ctypes.c_size_t]
    lib.axon_start_nrt_profile.restype = ctypes.c_int64
    lib.axon_stop_nrt_profile.argtypes = [ctypes.c_char_p]
    lib.axon_stop_nrt_profile.restype = ctypes.c_int64

    @contextlib.contextmanager
    def _hook(output_dir, device_ids):
        import jax

        jax.devices()
        if device_ids:
            ids = (ctypes.c_int64 * len(device_ids))(*device_ids)
            rc = lib.axon_start_nrt_profile(ids, len(device_ids))
        else:
            rc = lib.axon_start_nrt_profile(None, 0)
        if rc != 0:
            raise RuntimeError(f"axon_start_nrt_profile rc={rc}")
        try:
            yield
        finally:
            n = lib.axon_stop_nrt_profile(str(output_dir).encode())
            print(f"profile: {n} file(s) written to {output_dir}",
                  file=sys.stderr)

    set_axon_ntff_profile_hook(_hook)
    return True


def kernel(a_pad, b_pad, len_a, len_b, Wq, bq, Wk, bk, Wv, bv):
    global LAST_EXEC_TIME_NS
    import ml_dtypes
    BF = ml_dtypes.bfloat16

    a_pad = np.ascontiguousarray(np.asarray(a_pad, np.float32))
    b_pad = np.ascontiguousarray(np.asarray(b_pad, np.float32))
    len_a = np.asarray(len_a, np.int32)
    len_b = np.asarray(len_b, np.int32)
    Wq = np.asarray(Wq, np.float32)
    Wk = np.asarray(Wk, np.float32)
    Wv = np.asarray(Wv, np.float32)
    bq = np.asarray(bq, np.float32)
    bk = np.asarray(bk, np.float32)
    bv = np.asarray(bv, np.float32)

    swap, qa_len, qb_len, groups, slot_at, slot_bt = _plan(len_a, len_b)
    tot_at, tot_bt = sum(slot_at), sum(slot_bt)
    cum_at = np.concatenate([[0], np.cumsum(slot_at)]).astype(int)
    cum_bt = np.concatenate([[0], np.cumsum(slot_bt)]).astype(int)
    tot = tot_at + tot_bt
    cum = np.concatenate([[0], np.cumsum(
        [slot_at[s] + slot_bt[s] for s in range(NSLOTS)])]).astype(int)
    NSM = tot_at + tot_bt + 2 * NSLOTS + 2 * MI

    # ---- shared (per-core-identical) inputs ----
    # 1/sqrt(INNER) applied via the exp activation's scale argument, so q
    # stays ~N(0,1) for fp8 storage
    F8 = ml_dtypes.float8_e4m3fn
    wq_h = Wq.reshape(DT, P, INNER).transpose(1, 0, 2).astype(F8)
    wk_h = Wk.reshape(DT, P, INNER).transpose(1, 0, 2).astype(F8)
    wv_h = Wv.reshape(DT, P, OUTER).transpose(1, 0, 2).astype(BF)
    bqs_h = bq.reshape(MI, P).T.copy()
    bk_h = bk.reshape(MI, P).T.copy()
    bvb_h = np.broadcast_to(bv, (2 * NSLOTS, OUTER)).copy()
    idr_h = np.eye(P, dtype=np.float32)

    a16 = a_pad.astype(BF)
    b16 = b_pad.astype(BF)

    # ---- per-core inputs ----
    in_maps = []
    for c in range(NCORES):
        nat = np.zeros((tot * P, DIM), BF)
        tr = np.zeros((P, DT, tot * P), F8)
        sm = np.zeros((P, NSM), np.float32)
        gs_a = sm[:, 0:tot_at]
        gs_b = sm[:, tot_at:tot_at + tot_bt]
        npa = sm[:, tot_at + tot_bt:tot_at + tot_bt + NSLOTS]
        npb = sm[:, tot_at + tot_bt + NSLOTS:tot_at + tot_bt + 2 * NSLOTS]
        sm[:, NSM - 2 * MI:NSM - MI] = bqs_h
        sm[:, NSM - MI:NSM] = bk_h
        for s in range(NSLOTS):
            i = groups[s][c]
            la_i, lb_i = int(qa_len[i]), int(qb_len[i])
            A = b16[i] if swap[i] else a16[i]
            Bm = a16[i] if swap[i] else b16[i]
            ao = cum[s] * P                  # A rows/cols at slot start
            bo = (cum[s] + slot_at[s]) * P   # B rows/cols after A's
            nat[ao:ao + la_i] = A[:la_i]
            nat[bo:bo + lb_i] = Bm[:lb_i]
            # transposed layout: [dpart, dt, seq]
            tr[:, :, ao:ao + la_i] = \
                A[:la_i].T.reshape(DT, P, la_i).transpose(1, 0, 2)
            tr[:, :, bo:bo + lb_i] = \
                Bm[:lb_i].T.reshape(DT, P, lb_i).transpose(1, 0, 2)
            # g columns NEGATED (sign trick pairs with den_f = pc - den)
            ga = np.zeros(slot_at[s] * P, np.float32)
            ga[:la_i] = -1.0 / la_i
            gs_a[:, cum_at[s]:cum_at[s] + slot_at[s]] = \
                ga.reshape(slot_at[s], P).T
            gb = np.zeros(slot_bt[s] * P, np.float32)
            gb[:lb_i] = -1.0 / lb_i
            gs_b[:, cum_bt[s]:cum_bt[s] + slot_bt[s]] = \
                gb.reshape(slot_bt[s], P).T
            na_i = slot_at[s] * P - la_i
            nb_i = slot_bt[s] * P - lb_i
            npa[:, s] = np.log(na_i) if na_i > 0 else -1e30
            npb[:, s] = np.log(nb_i) if nb_i > 0 else -1e30
        in_maps.append({
            "nat": nat, "tr": tr, "sm": sm,
            "wq": wq_h, "wk": wk_h, "wv": wv_h,
            "bvb": bvb_h, "idr": idr_h,
        })

    nc = _build_program(slot_at, slot_bt)

    from concourse.bass_utils import run_bass_kernel_spmd

    trace = os.environ.get("BASS_KERNEL_TRACE", "0") == "1"
    if trace:
        _install_profhook()
    res = run_bass_kernel_spmd(nc, in_maps, list(range(NCORES)), trace=trace)
    LAST_EXEC_TIME_NS = res.exec_time_ns

    emb_a = np.zeros((B, OUTER), np.float32)
    emb_b = np.zeros((B, OUTER), np.float32)
    for c in range(NCORES):
        e = np.asarray(res.results[c]["emb"], np.float32)
        for s in range(NSLOTS):
            i = groups[s][c]
            ea, eb = e[2 * s], e[2 * s + 1]  # A-queries, B-queries
            if swap[i]:
                emb_a[i], emb_b[i] = eb, ea
            else:
                emb_a[i], emb_b[i] = ea, eb
    return emb_a, emb_b


# revision 74
# speedup vs baseline: 1.7098x; 1.1807x over previous
"""Ragged cross-attention pooling kernel for Trainium2 (8 NeuronCores, SPMD).

Math (per pair, direction "A attends over B"):
    qa = (A @ Wq*scale + bq*scale)      [la, INNER]
    kb =  B @ Wk + bk                   [lb, INNER]
    s  = qa @ kb^T                      [la, lb]
    p  = exp(s)               (no max-subtraction needed: |s| <~ 6)
    den[q] = sum_k p[q, k]  (pad-corrected: all pad cols share p[:, -1])
    gcol[q] = valid(q) / (la * den[q])
    w[k] = sum_q gcol[q] p[q, k]        <- collapses the mean over queries
    emb  = (w^T B) @ Wv + bv            <- collapses attn@V and the V projection

v2: A/B pre-transposed ON HOST (no on-chip transposes), all matmul inputs
bf16 (1 cyc/row at any moving size), single wide exp per query tile into a
2-bank PSUM tile, w row->col via SBUF-to-SBUF scatter DMA, final E computed
as E^T = U^T Wv with 16-wide stationary.

Distribution: 64 pairs -> 8 slots x 8 cores (one shared SPMD program, shapes
fixed per slot to the max over cores; pairs bin-packed by length so padding is
small).
"""

import os
import sys

sys.path.insert(0, "/opt/trn_rl_repo")

import numpy as np

B, LA, LB, DIM, INNER, OUTER = 64, 1024, 1024, 640, 256, 1024
NCORES, NSLOTS, P = 8, 8, 128
SCALE = 1.0 / np.sqrt(INNER)
DT = DIM // P  # 5 d-chunks
MI = INNER // P  # 2 inner-chunks

LAST_EXEC_TIME_NS = None


def _chunks(total, cap=512):
    out, off = [], 0
    while off < total:
        c = min(cap, total - off)
        out.append((off, c))
        off += c
    return out


def _plan(la_all, lb_all):
    """Assign pairs to (slot, core); returns swap flags, groups, slot tile shapes."""
    la = np.asarray(la_all, np.int64)
    lb = np.asarray(lb_all, np.int64)
    swap = lb > la
    qa = np.where(swap, lb, la)  # kernel A-side length (>= B-side)
    qb = np.where(swap, la, lb)
    at = -(-qa // P)
    bt = -(-qb // P)
    order = np.argsort(-(at * 1024 + bt), kind="stable")
    groups = [list(order[s * NCORES:(s + 1) * NCORES]) for s in range(NSLOTS)]
    C1, C2 = 1000.0, 450.0

    def gcost(g):
        ma = max(at[i] for i in g)
        mb = max(bt[i] for i in g)
        return C1 * (ma + mb) + C2 * ma * mb

    rng = np.random.default_rng(0)
    cost = [gcost(g) for g in groups]
    NIT = 120000
    s1s = rng.integers(0, NSLOTS, NIT)
    s2s = rng.integers(0, NSLOTS, NIT)
    i1s = rng.integers(0, NCORES, NIT)
    i2s = rng.integers(0, NCORES, NIT)
    for s1, s2, i1, i2 in zip(s1s, s2s, i1s, i2s):
        if s1 == s2:
            continue
        g1 = groups[s1][:]
        g2 = groups[s2][:]
        g1[i1], g2[i2] = groups[s2][i2], groups[s1][i1]
        n1, n2 = gcost(g1), gcost(g2)
        if n1 + n2 < cost[s1] + cost[s2] - 1e-9:
            groups[s1], groups[s2] = g1, g2
            cost[s1], cost[s2] = n1, n2
    slot_at = [max(at[i] for i in g) for g in groups]
    slot_bt = [max(bt[i] for i in g) for g in groups]
    # small slots first (minimizes the pipeline-fill bubble), except the
    # smallest goes LAST: the final slot's attention has no next-slot
    # projection filler, so keep it short
    sorder = sorted(range(NSLOTS), key=lambda s: cost[s])
    sorder = sorder[1:] + sorder[:1]
    groups = [groups[s] for s in sorder]
    slot_at = [slot_at[s] for s in sorder]
    slot_bt = [slot_bt[s] for s in sorder]
    return swap, qa, qb, groups, slot_at, slot_bt


def _build_program(slot_at, slot_bt):
    import concourse.bass as bass  # noqa: F401
    import concourse.mybir as mybir
    import concourse.tile as tile
    from concourse.tile import add_dep_helper
    from concourse import bacc

    F32 = mybir.dt.float32
    F32R = mybir.dt.float32r
    BF16 = mybir.dt.bfloat16
    FP8 = mybir.dt.float8e4
    DR = mybir.MatmulPerfMode.DoubleRow
    Exp = mybir.ActivationFunctionType.Exp
    Ident = mybir.ActivationFunctionType.Identity

    tot_at = sum(slot_at)
    tot_bt = sum(slot_bt)
    cum_at = np.concatenate([[0], np.cumsum(slot_at)]).astype(int)
    cum_bt = np.concatenate([[0], np.cumsum(slot_bt)]).astype(int)

    nc = bacc.Bacc("TRN2", target_bir_lowering=False, debug=False,
                   num_devices=NCORES)

    tot = tot_at + tot_bt
    # natural layout (row-tiled; per slot A-rows then B-rows) for u = w^T B
    nat_d = nc.dram_tensor("nat", [tot * P, DIM], BF16, kind="ExternalInput")
    # host-transposed layout [dpart, dt, seq] (per slot A-cols then B-cols);
    # fp8 so projections run DoubleRow
    tr_d = nc.dram_tensor("tr", [P, DT, tot * P], FP8, kind="ExternalInput")
    # all small per-core constants packed into one tensor:
    # [gs_a | gs_b | npa | npb | bqs | bk] along the free dim
    NSM = tot_at + tot_bt + NSLOTS + NSLOTS + MI + MI
    sm_d = nc.dram_tensor("sm", [P, NSM], F32, kind="ExternalInput")
    wq_d = nc.dram_tensor("wq", [P, DT, INNER], FP8, kind="ExternalInput")
    wk_d = nc.dram_tensor("wk", [P, DT, INNER], FP8, kind="ExternalInput")
    wv_d = nc.dram_tensor("wv", [P, DT, OUTER], BF16, kind="ExternalInput")
    bvb_d = nc.dram_tensor("bvb", [2 * NSLOTS, OUTER], F32,
                           kind="ExternalInput")
    idr_d = nc.dram_tensor("idr", [P, P], F32R, kind="ExternalInput")
    emb_d = nc.dram_tensor("emb", [2 * NSLOTS, OUTER], F32,
                           kind="ExternalOutput")
    # DRAM bounce buffer for the w row->col partition scatter
    wsc_d = nc.dram_tensor("wsc", [2 * NSLOTS, NCORES * P], BF16,
                           kind="ExternalOutput")
    cum = np.concatenate([[0], np.cumsum(
        [slot_at[s] + slot_bt[s] for s in range(NSLOTS)])]).astype(int)

    with tile.TileContext(nc) as tc:
        with (
            tc.tile_pool(name="const", bufs=1) as cpool,
            tc.tile_pool(name="anat", bufs=2) as apool,
            tc.tile_pool(name="atr", bufs=2) as atpool,
            tc.tile_pool(name="proj", bufs=3) as ppool,
            tc.tile_pool(name="pexp", bufs=3) as epool,
            tc.tile_pool(name="small", bufs=4) as spool,
            tc.tile_pool(name="late", bufs=2) as lpool,
            tc.tile_pool(name="psB", bufs=3, space="PSUM") as psB,
            tc.tile_pool(name="psW", bufs=1, space="PSUM") as psW,
        ):
            # ---- constants ----
            wq_sb = cpool.tile([P, DT, INNER], FP8, tag="wq")
            wk_sb = cpool.tile([P, DT, INNER], FP8, tag="wk")
            wv_sb = cpool.tile([P, DT, OUTER], BF16, tag="wv")
            bvb_sb = cpool.tile([2 * NSLOTS, OUTER], F32, tag="bvb")
            idr_sb = cpool.tile([P, P], F32R, tag="idr")
            sm_sb = cpool.tile([P, NSM], F32, tag="sm")
            # column offsets into sm_sb: [gs_a | gs_b | npa | npb | bqs | bk]
            GA, GB = 0, tot_at
            NPA, NPB = tot_at + tot_bt, tot_at + tot_bt + NSLOTS
            BQ, BK = NSM - 2 * MI, NSM - MI
            urows_sb = cpool.tile([2 * NSLOTS, DIM], F32R, tag="urows")
            idb2_sb = cpool.tile([1, 2], BF16, tag="idb2")
            nc.vector.memset(idb2_sb[0:1, 0:1], 1.0)
            nc.vector.memset(idb2_sb[0:1, 1:2], 0.0)

            ev = [0]  # evac engine alternator (shared across closures)

            def emit_loads(s):
                at_s, bt_s = int(slot_at[s]), int(slot_bt[s])
                nt = at_s + bt_s
                tr_sb = atpool.tile([P, DT, nt * P], FP8, tag="tr")
                nc.sync.dma_start(
                    tr_sb[:], tr_d[:, :, cum[s] * P:cum[s + 1] * P])
                if s == 0:
                    # slot-0 inputs are already in flight; issue the rest
                    # from other engine queues so DGE setups overlap
                    nc.scalar.dma_start(wq_sb[:], wq_d[:])
                    nc.gpsimd.dma_start(wk_sb[:], wk_d[:])
                    nc.gpsimd.dma_start(sm_sb[:], sm_d[:])
                nat_sb = apool.tile([P, nt, DIM], BF16, tag="nat")
                nc.sync.dma_start(
                    nat_sb[:], nat_d[cum[s] * P:cum[s + 1] * P, :]
                    .rearrange("(t p) d -> p t d", p=P))
                return tr_sb, nat_sb

            def proj_blocks(s, tr_sb):
                """Allocate the q/k tiles for slot s and return (tiles,
                thunks); each thunk emits one (projection, m) block.
                q/k stored fp8-e4m3: scores then run one DoubleRow matmul
                per chunk (0.5 cyc/row, all 256 contraction at once)."""
                at_s, bt_s = int(slot_at[s]), int(slot_bt[s])
                pla, plb = at_s * P, bt_s * P
                qaT = ppool.tile([P, MI, pla], FP8, tag="qaT")
                kaT = ppool.tile([P, MI, pla], FP8, tag="kaT")
                qbT = ppool.tile([P, MI, plb], FP8, tag="qbT")
                kbT = ppool.tile([P, MI, plb], FP8, tag="kbT")

                def chunk_blk(hold, ci, nch, dst, soff, pl, w_sb, bo, m):
                    if ci == 0:
                        hold['pp'] = psB.tile([P, 1024], F32, tag="big",
                                              name="pp")
                    pp = hold['pp']
                    noff, nlen = _chunks(pl)[ci]
                    # DT=5 contraction tiles: 2 DoubleRow pair matmuls
                    # + 1 plain fp8 matmul
                    for g in range(2):
                        nc.tensor.matmul(
                            pp[:, noff:noff + nlen],
                            w_sb[:, 2 * g:2 * g + 2, m * P:(m + 1) * P],
                            tr_sb[:, 2 * g:2 * g + 2,
                                  soff + noff:soff + noff + nlen],
                            start=(g == 0), stop=False, perf_mode=DR)
                    nc.tensor.matmul(
                        pp[:, noff:noff + nlen],
                        w_sb[:, DT - 1, m * P:(m + 1) * P],
                        tr_sb[:, DT - 1,
                              soff + noff:soff + noff + nlen],
                        start=False, stop=True)
                    if ci == nch - 1:
                        # vector gets 2 of 3 evacs (it has more slack)
                        if ev[0] % 3 != 2:
                            nc.vector.tensor_scalar_add(
                                dst[:, m, :], pp[:, :pl],
                                sm_sb[:, bo + m, None])
                        else:
                            nc.scalar.activation(
                                dst[:, m, :], pp[:, :pl], Ident,
                                bias=sm_sb[:, bo + m, None], scale=1.0)
                        ev[0] += 1

                thunks = []
                for dst, soff, pl, w_sb, bo in (
                        (qaT, 0, pla, wq_sb, BQ),
                        (kbT, pla, plb, wk_sb, BK),
                        (kaT, 0, pla, wk_sb, BK),
                        (qbT, pla, plb, wq_sb, BQ)):
                    for m in range(MI):
                        hold = {}
                        nch = len(_chunks(pl))
                        for ci in range(nch):
                            thunks.append(
                                lambda hold=hold, ci=ci, nch=nch, dst=dst,
                                soff=soff, pl=pl, w_sb=w_sb, bo=bo, m=m:
                                chunk_blk(hold, ci, nch, dst, soff, pl,
                                          w_sb, bo, m))
                return (qaT, kaT, qbT, kbT), thunks

            def atten_parts(s, tiles, nat_sb):
                """Thunks for slot s's attention: one per query tile plus a
                tail per direction.  Direction A's tail is interleaved just
                after direction B's first query tile so its cross-engine
                chain hides behind PE work."""
                at_s, bt_s = int(slot_at[s]), int(slot_bt[s])
                qaT, kaT, qbT, kbT = tiles
                dir_parts = []
                for dr in range(2):
                    if dr == 0:  # A queries over B keys
                        QT, KT, nq, nk = qaT, kbT, at_s, bt_s
                        g_off = GA + cum_at[s]
                        np_off = NPB + s
                        koff = at_s  # B rows sit after A rows in nat_sb
                    else:
                        QT, KT, nq, nk = qbT, kaT, bt_s, at_s
                        g_off = GB + cum_bt[s]
                        np_off = NPA + s
                        koff = 0
                    plk = nk * P
                    kchunks = _chunks(plk)
                    # w accumulator in its own 2-bank pool; allocated lazily
                    # by the first query tile
                    wrh = {}

                    def qt_part(qt, QT, KT, nq, nk, plk, kchunks, wrh,
                                g_off, np_off):
                        if qt == 0:
                            wrh['t'] = [
                                psW.tile([1, cl], F32, tag=f"wr{ci}",
                                         name=f"wr{ci}")
                                for ci, (co, cl) in enumerate(kchunks)]
                        wrt = wrh['t']
                        sc = psB.tile([P, 1024], F32, tag="big")
                        for co, cl in kchunks:
                            nc.tensor.matmul(
                                sc[:, co:co + cl],
                                QT[:, :, qt * P:(qt + 1) * P],
                                KT[:, :, co:co + cl],
                                start=True, stop=True, perf_mode=DR)
                        # pad contribution npad*p_pad as exp(s_pad+ln(npad))
                        # in f32 (bf16 p_pad would amplify through the
                        # den - npad*p_pad cancellation); sm holds ln(npad).
                        pc = spool.tile([P, 1], F32, tag="pc")
                        nc.scalar.activation(
                            pc[:], sc[:, plk - 1:plk], Exp,
                            bias=sm_sb[:, np_off, None], scale=SCALE)
                        den = spool.tile([P, 1], F32, tag="den")
                        p_sb = epool.tile([P, plk], BF16, tag="p_sb")
                        nc.scalar.activation(
                            p_sb[:], sc[:, :plk], Exp,
                            bias=0.0, scale=SCALE, accum_out=den[:])
                        # den_f = pad - den  (= -true_den; g is negated on
                        # host so gcol comes out positive)
                        denf = spool.tile([P, 1], F32, tag="denf")
                        nc.vector.tensor_sub(denf[:], pc[:], den[:])
                        rec = spool.tile([P, 1], F32, tag="rec")
                        nc.vector.reciprocal(rec[:], denf[:])
                        gcol = spool.tile([P, 1], BF16, tag="gcol")
                        nc.vector.tensor_mul(gcol[:], rec[:],
                                             sm_sb[:, g_off + qt, None])
                        for ci, (co, cl) in enumerate(kchunks):
                            nc.tensor.matmul(
                                wrt[ci][:], gcol[:],
                                p_sb[:, co:co + cl],
                                start=(qt == 0), stop=(qt == nq - 1))

                    def tail(nk, plk, kchunks, wrh, koff, row):
                        wrt = wrh['t']
                        # w row -> w col (transpose via identity matmuls;
                        # 2-wide output keeps PSUM writes 8B-aligned)
                        wrow = lpool.tile([1, plk], BF16, tag="wrow")
                        for ci, (co, cl) in enumerate(kchunks):
                            nc.vector.tensor_copy(wrow[0:1, co:co + cl],
                                                  wrt[ci][:])
                        wt = psB.tile([P, 1024], F32, tag="big")
                        for kt in range(nk):
                            nc.tensor.matmul(
                                wt[:, 2 * kt:2 * kt + 2],
                                wrow[0:1, kt * P:(kt + 1) * P],
                                idb2_sb[0:1, 0:2], start=True, stop=True)
                        wcol = lpool.tile([P, nk], BF16, tag="wcol")
                        nc.vector.tensor_copy(
                            wcol[:],
                            wt[:, :2 * nk].rearrange(
                                "p (k two) -> p k two", two=2)[:, :, 0])
                        # u row = w^T @ Knat
                        ur = psB.tile([P, 1024], F32, tag="big")
                        for noff, nlen in _chunks(DIM):
                            for kt in range(nk):
                                nc.tensor.matmul(
                                    ur[0:1, noff:noff + nlen],
                                    wcol[:, kt:kt + 1],
                                    nat_sb[:, koff + kt, noff:noff + nlen],
                                    start=(kt == 0), stop=(kt == nk - 1))
                        ursb = lpool.tile([1, DIM], F32R, tag="ursb")
                        nc.vector.tensor_copy(ursb[:].bitcast(F32),
                                              ur[0:1, :DIM])
                        nc.sync.dma_start(
                            urows_sb[row:row + 1, :], ursb[:])

                    qts = [
                        lambda qt=qt, QT=QT, KT=KT, nq=nq, nk=nk,
                        plk=plk, kchunks=kchunks, wrh=wrh, g_off=g_off,
                        np_off=np_off: qt_part(qt, QT, KT, nq, nk, plk,
                                               kchunks, wrh, g_off,
                                               np_off)
                        for qt in range(nq)]
                    tl = (lambda nk=nk, plk=plk, kchunks=kchunks, wrh=wrh,
                          koff=koff, row=2 * s + dr:
                          tail(nk, plk, kchunks, wrh, koff, row))
                    dir_parts.append((qts, tl))
                (qts_a, tl_a), (qts_b, tl_b) = dir_parts
                # A's tail goes right after B's first query tile
                parts = qts_a + qts_b[:1] + [tl_a] + qts_b[1:] + [tl_b]
                return parts

            # ---- software-pipelined emission: slot s+1's projections are
            # interleaved between slot s's attention parts so the PE always
            # has independent work during the exp/normalize latencies ----
            tr0, nat0 = emit_loads(0)
            tiles_cur, thunks0 = proj_blocks(0, tr0)
            for t in thunks0:
                t()
            nat_cur = nat0
            for s in range(NSLOTS):
                if s + 1 < NSLOTS:
                    tr_n, nat_n = emit_loads(s + 1)
                    tiles_n, blocks = proj_blocks(s + 1, tr_n)
                else:
                    tiles_n = nat_n = None
                    blocks = []
                parts = atten_parts(s, tiles_cur, nat_cur)
                nb, npt = len(blocks), len(parts)
                bi = 0
                for i, part in enumerate(parts):
                    part()
                    want = (i + 1) * nb // npt
                    while bi < want:
                        blocks[bi]()
                        bi += 1
                while bi < nb:
                    blocks[bi]()
                    bi += 1
                tiles_cur, nat_cur = tiles_n, nat_n

            # ---- final: E^T = U^T Wv + bv ----
            for sb, d in ((wv_sb, wv_d), (bvb_sb, bvb_d), (idr_sb, idr_d)):
                nc.sync.dma_start(sb[:], d[:])
            u_sb = cpool.tile([P, DT, 2 * NSLOTS], BF16, tag="usb")
            for dt in range(DT):
                ut = psB.tile([P, 1024], F32, tag="big")
                nc.tensor.matmul(
                    ut[:, :2 * NSLOTS],
                    urows_sb[:, dt * P:(dt + 1) * P],
                    idr_sb[0:2 * NSLOTS, 0:2 * NSLOTS],
                    start=True, stop=True)
                nc.vector.tensor_copy(u_sb[:, dt, :], ut[:, :2 * NSLOTS])
            eT = psB.tile([P, 1024], F32, tag="big")
            for noff, nlen in _chunks(OUTER):
                for dt in range(DT):
                    nc.tensor.matmul(
                        eT[0:2 * NSLOTS, noff:noff + nlen],
                        u_sb[:, dt, :],
                        wv_sb[:, dt, noff:noff + nlen],
                        start=(dt == 0), stop=(dt == DT - 1))
            e_sb = cpool.tile([2 * NSLOTS, OUTER], F32, tag="esb")
            nc.vector.tensor_add(e_sb[:], eT[0:2 * NSLOTS, :], bvb_sb[:])
            nc.sync.dma_start(emb_d[:], e_sb[:])

    nc.compile()
    return nc


def _install_profhook():
    import contextlib
    import ctypes
    import types

    import antenv

    if not hasattr(antenv, "axon_hooks"):
        mod = types.ModuleType("antenv.axon_hooks")
        mod._hook = None

        def _set(h):
            mod._hook = h

        def _get():
            return mod._hook

        mod.set_axon_ntff_profile_hook = _set
        mod.get_axon_ntff_profile_hook = _get
        sys.modules["antenv.axon_hooks"] = mod
        antenv.axon_hooks = mod
    from antenv.axon_hooks import set_axon_ntff_profile_hook
    so_path = "/opt/axon/libaxon_pjrt.so"
    if not os.path.exists(so_path):
        return False
    lib = ctypes.CDLL(so_path)
    if not hasattr(lib, "axon_start_nrt_profile"):
        return False
    lib.axon_start_nrt_profile.argtypes = [ctypes.POINTER(ctypes.c_int64),
                                           ctypes.c_size_t]
    lib.axon_start_nrt_profile.restype = ctypes.c_int64
    lib.axon_stop_nrt_profile.argtypes = [ctypes.c_char_p]
    lib.axon_stop_nrt_profile.restype = ctypes.c_int64

    @contextlib.contextmanager
    def _hook(output_dir, device_ids):
        import jax

        jax.devices()
        if device_ids:
            ids = (ctypes.c_int64 * len(device_ids))(*device_ids)
            rc = lib.axon_start_nrt_profile(ids, len(device_ids))
        else:
            rc = lib.axon_start_nrt_profile(None, 0)
        if rc != 0:
            raise RuntimeError(f"axon_start_nrt_profile rc={rc}")
        try:
            yield
        finally:
            n = lib.axon_stop_nrt_profile(str(output_dir).encode())
            print(f"profile: {n} file(s) written to {output_dir}",
                  file=sys.stderr)

    set_axon_ntff_profile_hook(_hook)
    return True


def kernel(a_pad, b_pad, len_a, len_b, Wq, bq, Wk, bk, Wv, bv):
    global LAST_EXEC_TIME_NS
    import ml_dtypes
    BF = ml_dtypes.bfloat16

    a_pad = np.ascontiguousarray(np.asarray(a_pad, np.float32))
    b_pad = np.ascontiguousarray(np.asarray(b_pad, np.float32))
    len_a = np.asarray(len_a, np.int32)
    len_b = np.asarray(len_b, np.int32)
    Wq = np.asarray(Wq, np.float32)
    Wk = np.asarray(Wk, np.float32)
    Wv = np.asarray(Wv, np.float32)
    bq = np.asarray(bq, np.float32)
    bk = np.asarray(bk, np.float32)
    bv = np.asarray(bv, np.float32)

    swap, qa_len, qb_len, groups, slot_at, slot_bt = _plan(len_a, len_b)
    tot_at, tot_bt = sum(slot_at), sum(slot_bt)
    cum_at = np.concatenate([[0], np.cumsum(slot_at)]).astype(int)
    cum_bt = np.concatenate([[0], np.cumsum(slot_bt)]).astype(int)
    tot = tot_at + tot_bt
    cum = np.concatenate([[0], np.cumsum(
        [slot_at[s] + slot_bt[s] for s in range(NSLOTS)])]).astype(int)
    NSM = tot_at + tot_bt + 2 * NSLOTS + 2 * MI

    # ---- shared (per-core-identical) inputs ----
    # 1/sqrt(INNER) applied via the exp activation's scale argument, so q
    # stays ~N(0,1) for fp8 storage
    F8 = ml_dtypes.float8_e4m3fn
    wq_h = Wq.reshape(DT, P, INNER).transpose(1, 0, 2).astype(F8)
    wk_h = Wk.reshape(DT, P, INNER).transpose(1, 0, 2).astype(F8)
    wv_h = Wv.reshape(DT, P, OUTER).transpose(1, 0, 2).astype(BF)
    bqs_h = bq.reshape(MI, P).T.copy()
    bk_h = bk.reshape(MI, P).T.copy()
    bvb_h = np.broadcast_to(bv, (2 * NSLOTS, OUTER)).copy()
    idr_h = np.eye(P, dtype=np.float32)

    a16 = a_pad.astype(BF)
    b16 = b_pad.astype(BF)

    # ---- per-core inputs ----
    in_maps = []
    for c in range(NCORES):
        nat = np.zeros((tot * P, DIM), BF)
        tr = np.zeros((P, DT, tot * P), F8)
        sm = np.zeros((P, NSM), np.float32)
        gs_a = sm[:, 0:tot_at]
        gs_b = sm[:, tot_at:tot_at + tot_bt]
        npa = sm[:, tot_at + tot_bt:tot_at + tot_bt + NSLOTS]
        npb = sm[:, tot_at + tot_bt + NSLOTS:tot_at + tot_bt + 2 * NSLOTS]
        sm[:, NSM - 2 * MI:NSM - MI] = bqs_h
        sm[:, NSM - MI:NSM] = bk_h
        for s in range(NSLOTS):
            i = groups[s][c]
            la_i, lb_i = int(qa_len[i]), int(qb_len[i])
            A = b16[i] if swap[i] else a16[i]
            Bm = a16[i] if swap[i] else b16[i]
            ao = cum[s] * P                  # A rows/cols at slot start
            bo = (cum[s] + slot_at[s]) * P   # B rows/cols after A's
            nat[ao:ao + la_i] = A[:la_i]
            nat[bo:bo + lb_i] = Bm[:lb_i]
            # transposed layout: [dpart, dt, seq]
            tr[:, :, ao:ao + la_i] = \
                A[:la_i].T.reshape(DT, P, la_i).transpose(1, 0, 2)
            tr[:, :, bo:bo + lb_i] = \
                Bm[:lb_i].T.reshape(DT, P, lb_i).transpose(1, 0, 2)
            # g columns NEGATED (sign trick pairs with den_f = pc - den)
            ga = np.zeros(slot_at[s] * P, np.float32)
            ga[:la_i] = -1.0 / la_i
            gs_a[:, cum_at[s]:cum_at[s] + slot_at[s]] = \
                ga.reshape(slot_at[s], P).T
            gb = np.zeros(slot_bt[s] * P, np.float32)
            gb[:lb_i] = -1.0 / lb_i
            gs_b[:, cum_bt[s]:cum_bt[s] + slot_bt[s]] = \
                gb.reshape(slot_bt[s], P).T
            na_i = slot_at[s] * P - la_i
            nb_i = slot_bt[s] * P - lb_i
            npa[:, s] = np.log(na_i) if na_i > 0 else -1e30
            npb[:, s] = np.log(nb_i) if nb_i > 0 else -1e30
        in_maps.append({
            "nat": nat, "tr": tr, "sm": sm,
            "wq": wq_h, "wk": wk_h, "wv": wv_h,
            "bvb": bvb_h, "idr": idr_h,
        })

    nc = _build_program(slot_at, slot_bt)

    from concourse.bass_utils import run_bass_kernel_spmd

    trace = os.environ.get("BASS_KERNEL_TRACE", "0") == "1"
    if trace:
        _install_profhook()
    res = run_bass_kernel_spmd(nc, in_maps, list(range(NCORES)), trace=trace)
    LAST_EXEC_TIME_NS = res.exec_time_ns

    emb_a = np.zeros((B, OUTER), np.float32)
    emb_b = np.zeros((B, OUTER), np.float32)
    for c in range(NCORES):
        e = np.asarray(res.results[c]["emb"], np.float32)
        for s in range(NSLOTS):
            i = groups[s][c]
            ea, eb = e[2 * s], e[2 * s + 1]  # A-queries, B-queries
            if swap[i]:
                emb_a[i], emb_b[i] = eb, ea
            else:
                emb_a[i], emb_b[i] = ea, eb
    return emb_a, emb_b


# revision 76
# speedup vs baseline: 1.7668x; 1.0333x over previous
"""Ragged cross-attention pooling kernel for Trainium2 (8 NeuronCores, SPMD).

Math (per pair, direction "A attends over B"):
    qa = (A @ Wq*scale + bq*scale)      [la, INNER]
    kb =  B @ Wk + bk                   [lb, INNER]
    s  = qa @ kb^T                      [la, lb]
    p  = exp(s)               (no max-subtraction needed: |s| <~ 6)
    den[q] = sum_k p[q, k]  (pad-corrected: all pad cols share p[:, -1])
    gcol[q] = valid(q) / (la * den[q])
    w[k] = sum_q gcol[q] p[q, k]        <- collapses the mean over queries
    emb  = (w^T B) @ Wv + bv            <- collapses attn@V and the V projection

v2: A/B pre-transposed ON HOST (no on-chip transposes), all matmul inputs
bf16 (1 cyc/row at any moving size), single wide exp per query tile into a
2-bank PSUM tile, w row->col via SBUF-to-SBUF scatter DMA, final E computed
as E^T = U^T Wv with 16-wide stationary.

Distribution: 64 pairs -> 8 slots x 8 cores (one shared SPMD program, shapes
fixed per slot to the max over cores; pairs bin-packed by length so padding is
small).
"""

import os
import sys

sys.path.insert(0, "/opt/trn_rl_repo")

import numpy as np

B, LA, LB, DIM, INNER, OUTER = 64, 1024, 1024, 640, 256, 1024
NCORES, NSLOTS, P = 8, 8, 128
SCALE = 1.0 / np.sqrt(INNER)
DT = DIM // P  # 5 d-chunks
MI = INNER // P  # 2 inner-chunks

LAST_EXEC_TIME_NS = None


def _chunks(total, cap=512):
    out, off = [], 0
    while off < total:
        c = min(cap, total - off)
        out.append((off, c))
        off += c
    return out


def _plan(la_all, lb_all):
    """Assign pairs to (slot, core); returns swap flags, groups, slot tile shapes."""
    la = np.asarray(la_all, np.int64)
    lb = np.asarray(lb_all, np.int64)
    swap = lb > la
    qa = np.where(swap, lb, la)  # kernel A-side length (>= B-side)
    qb = np.where(swap, la, lb)
    at = -(-qa // P)
    bt = -(-qb // P)
    order = np.argsort(-(at * 1024 + bt), kind="stable")
    groups = [list(order[s * NCORES:(s + 1) * NCORES]) for s in range(NSLOTS)]
    C1, C2 = 1000.0, 450.0

    def gcost(g):
        ma = max(at[i] for i in g)
        mb = max(bt[i] for i in g)
        return C1 * (ma + mb) + C2 * ma * mb

    rng = np.random.default_rng(0)
    cost = [gcost(g) for g in groups]
    # simulated annealing over pair swaps (greedy gets stuck ~5% above)
    NIT = 400000
    s1s = rng.integers(0, NSLOTS, NIT)
    s2s = rng.integers(0, NSLOTS, NIT)
    i1s = rng.integers(0, NCORES, NIT)
    i2s = rng.integers(0, NCORES, NIT)
    us = rng.random(NIT)
    T0 = 600.0
    best = ([g[:] for g in groups], sum(cost))
    for it, (s1, s2, i1, i2, u) in enumerate(zip(s1s, s2s, i1s, i2s, us)):
        if s1 == s2:
            continue
        g1 = groups[s1][:]
        g2 = groups[s2][:]
        g1[i1], g2[i2] = groups[s2][i2], groups[s1][i1]
        n1, n2 = gcost(g1), gcost(g2)
        dl = (n1 + n2) - (cost[s1] + cost[s2])
        T = T0 * (1.0 - it / NIT)
        if dl < -1e-9 or (T > 1e-6 and u < np.exp(-dl / T)):
            groups[s1], groups[s2] = g1, g2
            cost[s1], cost[s2] = n1, n2
            tot_c = sum(cost)
            if tot_c < best[1]:
                best = ([g[:] for g in groups], tot_c)
    groups = best[0]
    cost = [gcost(g) for g in groups]
    slot_at = [max(at[i] for i in g) for g in groups]
    slot_bt = [max(bt[i] for i in g) for g in groups]
    # small slots first (minimizes the pipeline-fill bubble), except the
    # smallest goes LAST: the final slot's attention has no next-slot
    # projection filler, so keep it short
    sorder = sorted(range(NSLOTS), key=lambda s: cost[s])
    sorder = sorder[1:] + sorder[:1]
    groups = [groups[s] for s in sorder]
    slot_at = [slot_at[s] for s in sorder]
    slot_bt = [slot_bt[s] for s in sorder]
    return swap, qa, qb, groups, slot_at, slot_bt


def _build_program(slot_at, slot_bt):
    import concourse.bass as bass  # noqa: F401
    import concourse.mybir as mybir
    import concourse.tile as tile
    from concourse.tile import add_dep_helper
    from concourse import bacc

    F32 = mybir.dt.float32
    F32R = mybir.dt.float32r
    BF16 = mybir.dt.bfloat16
    FP8 = mybir.dt.float8e4
    DR = mybir.MatmulPerfMode.DoubleRow
    Exp = mybir.ActivationFunctionType.Exp
    Ident = mybir.ActivationFunctionType.Identity

    tot_at = sum(slot_at)
    tot_bt = sum(slot_bt)
    cum_at = np.concatenate([[0], np.cumsum(slot_at)]).astype(int)
    cum_bt = np.concatenate([[0], np.cumsum(slot_bt)]).astype(int)

    nc = bacc.Bacc("TRN2", target_bir_lowering=False, debug=False,
                   num_devices=NCORES)

    tot = tot_at + tot_bt
    # natural layout (row-tiled; per slot A-rows then B-rows) for u = w^T B
    nat_d = nc.dram_tensor("nat", [tot * P, DIM], BF16, kind="ExternalInput")
    # host-transposed layout [dpart, dt, seq] (per slot A-cols then B-cols);
    # fp8 so projections run DoubleRow
    tr_d = nc.dram_tensor("tr", [P, DT, tot * P], FP8, kind="ExternalInput")
    # all small per-core constants packed into one tensor:
    # [gs_a | gs_b | npa | npb | bqs | bk] along the free dim
    NSM = tot_at + tot_bt + NSLOTS + NSLOTS + MI + MI
    sm_d = nc.dram_tensor("sm", [P, NSM], F32, kind="ExternalInput")
    wq_d = nc.dram_tensor("wq", [P, DT, INNER], FP8, kind="ExternalInput")
    wk_d = nc.dram_tensor("wk", [P, DT, INNER], FP8, kind="ExternalInput")
    wv_d = nc.dram_tensor("wv", [P, DT, OUTER], BF16, kind="ExternalInput")
    bvb_d = nc.dram_tensor("bvb", [2 * NSLOTS, OUTER], F32,
                           kind="ExternalInput")
    idr_d = nc.dram_tensor("idr", [P, P], F32R, kind="ExternalInput")
    emb_d = nc.dram_tensor("emb", [2 * NSLOTS, OUTER], F32,
                           kind="ExternalOutput")
    # DRAM bounce buffer for the w row->col partition scatter
    wsc_d = nc.dram_tensor("wsc", [2 * NSLOTS, NCORES * P], BF16,
                           kind="ExternalOutput")
    cum = np.concatenate([[0], np.cumsum(
        [slot_at[s] + slot_bt[s] for s in range(NSLOTS)])]).astype(int)

    with tile.TileContext(nc) as tc:
        with (
            tc.tile_pool(name="const", bufs=1) as cpool,
            tc.tile_pool(name="anat", bufs=2) as apool,
            tc.tile_pool(name="atr", bufs=2) as atpool,
            tc.tile_pool(name="proj", bufs=3) as ppool,
            tc.tile_pool(name="pexp", bufs=3) as epool,
            tc.tile_pool(name="small", bufs=4) as spool,
            tc.tile_pool(name="late", bufs=3) as lpool,
            tc.tile_pool(name="psB", bufs=3, space="PSUM") as psB,
            tc.tile_pool(name="psW", bufs=1, space="PSUM") as psW,
        ):
            # ---- constants ----
            wq_sb = cpool.tile([P, DT, INNER], FP8, tag="wq")
            wk_sb = cpool.tile([P, DT, INNER], FP8, tag="wk")
            wv_sb = cpool.tile([P, DT, OUTER], BF16, tag="wv")
            bvb_sb = cpool.tile([2 * NSLOTS, OUTER], F32, tag="bvb")
            idr_sb = cpool.tile([P, P], F32R, tag="idr")
            sm_sb = cpool.tile([P, NSM], F32, tag="sm")
            # column offsets into sm_sb: [gs_a | gs_b | npa | npb | bqs | bk]
            GA, GB = 0, tot_at
            NPA, NPB = tot_at + tot_bt, tot_at + tot_bt + NSLOTS
            BQ, BK = NSM - 2 * MI, NSM - MI
            urows_sb = cpool.tile([2 * NSLOTS, DIM], F32R, tag="urows")
            idb2_sb = cpool.tile([1, 2], BF16, tag="idb2")
            nc.vector.memset(idb2_sb[0:1, 0:1], 1.0)
            nc.vector.memset(idb2_sb[0:1, 1:2], 0.0)

            ev = [0]  # evac engine alternator (shared across closures)

            def emit_loads(s):
                at_s, bt_s = int(slot_at[s]), int(slot_bt[s])
                nt = at_s + bt_s
                tr_sb = atpool.tile([P, DT, nt * P], FP8, tag="tr")
                nc.sync.dma_start(
                    tr_sb[:], tr_d[:, :, cum[s] * P:cum[s + 1] * P])
                if s == 0:
                    # slot-0 inputs are already in flight; issue the rest
                    # from other engine queues so DGE setups overlap
                    nc.scalar.dma_start(wq_sb[:], wq_d[:])
                    nc.gpsimd.dma_start(wk_sb[:], wk_d[:])
                    nc.gpsimd.dma_start(sm_sb[:], sm_d[:])
                nat_sb = apool.tile([P, nt, DIM], BF16, tag="nat")
                nc.sync.dma_start(
                    nat_sb[:], nat_d[cum[s] * P:cum[s + 1] * P, :]
                    .rearrange("(t p) d -> p t d", p=P))
                return tr_sb, nat_sb

            def proj_blocks(s, tr_sb):
                """Allocate the q/k tiles for slot s and return (tiles,
                thunks); each thunk emits one (projection, m) block.
                q/k stored fp8-e4m3: scores then run one DoubleRow matmul
                per chunk (0.5 cyc/row, all 256 contraction at once)."""
                at_s, bt_s = int(slot_at[s]), int(slot_bt[s])
                pla, plb = at_s * P, bt_s * P
                qaT = ppool.tile([P, MI, pla], FP8, tag="qaT")
                kaT = ppool.tile([P, MI, pla], FP8, tag="kaT")
                qbT = ppool.tile([P, MI, plb], FP8, tag="qbT")
                kbT = ppool.tile([P, MI, plb], FP8, tag="kbT")

                def chunk_blk(hold, ci, nch, dst, soff, pl, w_sb, bo, m):
                    if ci == 0:
                        hold['pp'] = psB.tile([P, 1024], F32, tag="big",
                                              name="pp")
                    pp = hold['pp']
                    noff, nlen = _chunks(pl)[ci]
                    # DT=5 contraction tiles: 2 DoubleRow pair matmuls
                    # + 1 plain fp8 matmul
                    for g in range(2):
                        nc.tensor.matmul(
                            pp[:, noff:noff + nlen],
                            w_sb[:, 2 * g:2 * g + 2, m * P:(m + 1) * P],
                            tr_sb[:, 2 * g:2 * g + 2,
                                  soff + noff:soff + noff + nlen],
                            start=(g == 0), stop=False, perf_mode=DR)
                    nc.tensor.matmul(
                        pp[:, noff:noff + nlen],
                        w_sb[:, DT - 1, m * P:(m + 1) * P],
                        tr_sb[:, DT - 1,
                              soff + noff:soff + noff + nlen],
                        start=False, stop=True)
                    if ci == nch - 1:
                        # vector gets 2 of 3 evacs (it has more slack)
                        if ev[0] % 3 != 2:
                            nc.vector.tensor_scalar_add(
                                dst[:, m, :], pp[:, :pl],
                                sm_sb[:, bo + m, None])
                        else:
                            nc.scalar.activation(
                                dst[:, m, :], pp[:, :pl], Ident,
                                bias=sm_sb[:, bo + m, None], scale=1.0)
                        ev[0] += 1

                thunks = []
                for dst, soff, pl, w_sb, bo in (
                        (qaT, 0, pla, wq_sb, BQ),
                        (kbT, pla, plb, wk_sb, BK),
                        (kaT, 0, pla, wk_sb, BK),
                        (qbT, pla, plb, wq_sb, BQ)):
                    for m in range(MI):
                        hold = {}
                        nch = len(_chunks(pl))
                        for ci in range(nch):
                            thunks.append(
                                lambda hold=hold, ci=ci, nch=nch, dst=dst,
                                soff=soff, pl=pl, w_sb=w_sb, bo=bo, m=m:
                                chunk_blk(hold, ci, nch, dst, soff, pl,
                                          w_sb, bo, m))
                return (qaT, kaT, qbT, kbT), thunks

            def atten_parts(s, tiles, nat_sb):
                """Thunks for slot s's attention: one per query tile plus a
                tail per direction.  Direction A's tail is interleaved just
                after direction B's first query tile so its cross-engine
                chain hides behind PE work."""
                at_s, bt_s = int(slot_at[s]), int(slot_bt[s])
                qaT, kaT, qbT, kbT = tiles
                dir_parts = []
                for dr in range(2):
                    if dr == 0:  # A queries over B keys
                        QT, KT, nq, nk = qaT, kbT, at_s, bt_s
                        g_off = GA + cum_at[s]
                        np_off = NPB + s
                        koff = at_s  # B rows sit after A rows in nat_sb
                    else:
                        QT, KT, nq, nk = qbT, kaT, bt_s, at_s
                        g_off = GB + cum_bt[s]
                        np_off = NPA + s
                        koff = 0
                    plk = nk * P
                    kchunks = _chunks(plk)
                    # w accumulator in its own 2-bank pool; allocated lazily
                    # by the first query tile
                    wrh = {}

                    def qt_part(qt, QT, KT, nq, nk, plk, kchunks, wrh,
                                g_off, np_off):
                        if qt == 0:
                            wrh['t'] = [
                                psW.tile([1, cl], F32, tag=f"wr{ci}",
                                         name=f"wr{ci}")
                                for ci, (co, cl) in enumerate(kchunks)]
                        wrt = wrh['t']
                        sc = psB.tile([P, 1024], F32, tag="big")
                        for co, cl in kchunks:
                            nc.tensor.matmul(
                                sc[:, co:co + cl],
                                QT[:, :, qt * P:(qt + 1) * P],
                                KT[:, :, co:co + cl],
                                start=True, stop=True, perf_mode=DR)
                        # pad contribution npad*p_pad as exp(s_pad+ln(npad))
                        # in f32 (bf16 p_pad would amplify through the
                        # den - npad*p_pad cancellation); sm holds ln(npad).
                        pc = spool.tile([P, 1], F32, tag="pc")
                        nc.scalar.activation(
                            pc[:], sc[:, plk - 1:plk], Exp,
                            bias=sm_sb[:, np_off, None], scale=SCALE)
                        den = spool.tile([P, 1], F32, tag="den")
                        p_sb = epool.tile([P, plk], BF16, tag="p_sb")
                        nc.scalar.activation(
                            p_sb[:], sc[:, :plk], Exp,
                            bias=0.0, scale=SCALE, accum_out=den[:])
                        # den_f = pad - den  (= -true_den; g is negated on
                        # host so gcol comes out positive)
                        denf = spool.tile([P, 1], F32, tag="denf")
                        nc.vector.tensor_sub(denf[:], pc[:], den[:])
                        rec = spool.tile([P, 1], F32, tag="rec")
                        nc.vector.reciprocal(rec[:], denf[:])
                        gcol = spool.tile([P, 1], BF16, tag="gcol")
                        nc.vector.tensor_mul(gcol[:], rec[:],
                                             sm_sb[:, g_off + qt, None])
                        for ci, (co, cl) in enumerate(kchunks):
                            nc.tensor.matmul(
                                wrt[ci][:], gcol[:],
                                p_sb[:, co:co + cl],
                                start=(qt == 0), stop=(qt == nq - 1))

                    def tail(nk, plk, kchunks, wrh, koff, row):
                        wrt = wrh['t']
                        # w row -> w col (transpose via identity matmuls;
                        # 2-wide output keeps PSUM writes 8B-aligned)
                        wrow = lpool.tile([1, plk], BF16, tag="wrow")
                        for ci, (co, cl) in enumerate(kchunks):
                            nc.vector.tensor_copy(wrow[0:1, co:co + cl],
                                                  wrt[ci][:])
                        wt = psB.tile([P, 1024], F32, tag="big")
                        for kt in range(nk):
                            nc.tensor.matmul(
                                wt[:, 2 * kt:2 * kt + 2],
                                wrow[0:1, kt * P:(kt + 1) * P],
                                idb2_sb[0:1, 0:2], start=True, stop=True)
                        wcol = lpool.tile([P, nk], BF16, tag="wcol")
                        nc.vector.tensor_copy(
                            wcol[:],
                            wt[:, :2 * nk].rearrange(
                                "p (k two) -> p k two", two=2)[:, :, 0])
                        # u row = w^T @ Knat
                        ur = psB.tile([P, 1024], F32, tag="big")
                        for noff, nlen in _chunks(DIM):
                            for kt in range(nk):
                                nc.tensor.matmul(
                                    ur[0:1, noff:noff + nlen],
                                    wcol[:, kt:kt + 1],
                                    nat_sb[:, koff + kt, noff:noff + nlen],
                                    start=(kt == 0), stop=(kt == nk - 1))
                        ursb = lpool.tile([1, DIM], F32R, tag="ursb")
                        nc.vector.tensor_copy(ursb[:].bitcast(F32),
                                              ur[0:1, :DIM])
                        nc.sync.dma_start(
                            urows_sb[row:row + 1, :], ursb[:])

                    qts = [
                        lambda qt=qt, QT=QT, KT=KT, nq=nq, nk=nk,
                        plk=plk, kchunks=kchunks, wrh=wrh, g_off=g_off,
                        np_off=np_off: qt_part(qt, QT, KT, nq, nk, plk,
                                               kchunks, wrh, g_off,
                                               np_off)
                        for qt in range(nq)]
                    tl = (lambda nk=nk, plk=plk, kchunks=kchunks, wrh=wrh,
                          koff=koff, row=2 * s + dr:
                          tail(nk, plk, kchunks, wrh, koff, row))
                    dir_parts.append((qts, tl))
                (qts_a, tl_a), (qts_b, tl_b) = dir_parts
                # A's tail goes right after B's first query tile
                parts = qts_a + qts_b[:1] + [tl_a] + qts_b[1:] + [tl_b]
                return parts

            # ---- software-pipelined emission: slot s+1's projections are
            # interleaved between slot s's attention parts so the PE always
            # has independent work during the exp/normalize latencies ----
            tr0, nat0 = emit_loads(0)
            tiles_cur, thunks0 = proj_blocks(0, tr0)
            for t in thunks0:
                t()
            nat_cur = nat0
            for s in range(NSLOTS):
                if s + 1 < NSLOTS:
                    tr_n, nat_n = emit_loads(s + 1)
                    tiles_n, blocks = proj_blocks(s + 1, tr_n)
                else:
                    tiles_n = nat_n = None
                    blocks = []
                parts = atten_parts(s, tiles_cur, nat_cur)
                nb, npt = len(blocks), len(parts)
                bi = 0
                for i, part in enumerate(parts):
                    part()
                    want = (i + 1) * nb // npt
                    while bi < want:
                        blocks[bi]()
                        bi += 1
                while bi < nb:
                    blocks[bi]()
                    bi += 1
                tiles_cur, nat_cur = tiles_n, nat_n

            # ---- final: E^T = U^T Wv + bv ----
            for sb, d in ((wv_sb, wv_d), (bvb_sb, bvb_d), (idr_sb, idr_d)):
                nc.sync.dma_start(sb[:], d[:])
            u_sb = cpool.tile([P, DT, 2 * NSLOTS], BF16, tag="usb")
            for dt in range(DT):
                ut = psB.tile([P, 1024], F32, tag="big")
                nc.tensor.matmul(
                    ut[:, :2 * NSLOTS],
                    urows_sb[:, dt * P:(dt + 1) * P],
                    idr_sb[0:2 * NSLOTS, 0:2 * NSLOTS],
                    start=True, stop=True)
                nc.vector.tensor_copy(u_sb[:, dt, :], ut[:, :2 * NSLOTS])
            eT = psB.tile([P, 1024], F32, tag="big")
            for noff, nlen in _chunks(OUTER):
                for dt in range(DT):
                    nc.tensor.matmul(
                        eT[0:2 * NSLOTS, noff:noff + nlen],
                        u_sb[:, dt, :],
                        wv_sb[:, dt, noff:noff + nlen],
                        start=(dt == 0), stop=(dt == DT - 1))
            e_sb = cpool.tile([2 * NSLOTS, OUTER], F32, tag="esb")
            nc.vector.tensor_add(e_sb[:], eT[0:2 * NSLOTS, :], bvb_sb[:])
            nc.sync.dma_start(emb_d[:], e_sb[:])

    nc.compile()
    return nc


def _install_profhook():
    import contextlib
    import ctypes
    import types

    import antenv

    if not hasattr(antenv, "axon_hooks"):
        mod = types.ModuleType("antenv.axon_hooks")
        mod._hook = None

        def _set(h):
            mod._hook = h

        def _get():
            return mod._hook

        mod.set_axon_ntff_profile_hook = _set
        mod.get_axon_ntff_profile_hook = _get
        sys.modules["antenv.axon_hooks"] = mod
        antenv.axon_hooks = mod
    from antenv.axon_hooks import set_axon_ntff_profile_hook
    so_path = "/opt/axon/libaxon_pjrt.so"
    if not os.path.exists(so_path):
        return False
    lib = ctypes.CDLL(so_path)
    if not hasattr(lib, "axon_start_nrt_profile"):
        return False
    lib.axon_start_nrt_profile.argtypes = [ctypes.POINTER(ctypes.c_int64),
                                           ctypes.c_size_t]
    lib.axon_start_nrt_profile.restype = ctypes.c_int64
    lib.axon_stop_nrt_profile.argtypes = [ctypes.c_char_p]
    lib.axon_stop_nrt_profile.restype = ctypes.c_int64

    @contextlib.contextmanager
    def _hook(output_dir, device_ids):
        import jax

        jax.devices()
        if device_ids:
            ids = (ctypes.c_int64 * len(device_ids))(*device_ids)
            rc = lib.axon_start_nrt_profile(ids, len(device_ids))
        else:
            rc = lib.axon_start_nrt_profile(None, 0)
        if rc != 0:
            raise RuntimeError(f"axon_start_nrt_profile rc={rc}")
        try:
            yield
        finally:
            n = lib.axon_stop_nrt_profile(str(output_dir).encode())
            print(f"profile: {n} file(s) written to {output_dir}",
                  file=sys.stderr)

    set_axon_ntff_profile_hook(_hook)
    return True


def kernel(a_pad, b_pad, len_a, len_b, Wq, bq, Wk, bk, Wv, bv):
    global LAST_EXEC_TIME_NS
    import ml_dtypes
    BF = ml_dtypes.bfloat16

    a_pad = np.ascontiguousarray(np.asarray(a_pad, np.float32))
    b_pad = np.ascontiguousarray(np.asarray(b_pad, np.float32))
    len_a = np.asarray(len_a, np.int32)
    len_b = np.asarray(len_b, np.int32)
    Wq = np.asarray(Wq, np.float32)
    Wk = np.asarray(Wk, np.float32)
    Wv = np.asarray(Wv, np.float32)
    bq = np.asarray(bq, np.float32)
    bk = np.asarray(bk, np.float32)
    bv = np.asarray(bv, np.float32)

    swap, qa_len, qb_len, groups, slot_at, slot_bt = _plan(len_a, len_b)
    tot_at, tot_bt = sum(slot_at), sum(slot_bt)
    cum_at = np.concatenate([[0], np.cumsum(slot_at)]).astype(int)
    cum_bt = np.concatenate([[0], np.cumsum(slot_bt)]).astype(int)
    tot = tot_at + tot_bt
    cum = np.concatenate([[0], np.cumsum(
        [slot_at[s] + slot_bt[s] for s in range(NSLOTS)])]).astype(int)
    NSM = tot_at + tot_bt + 2 * NSLOTS + 2 * MI

    # ---- shared (per-core-identical) inputs ----
    # 1/sqrt(INNER) applied via the exp activation's scale argument, so q
    # stays ~N(0,1) for fp8 storage
    F8 = ml_dtypes.float8_e4m3fn
    wq_h = Wq.reshape(DT, P, INNER).transpose(1, 0, 2).astype(F8)
    wk_h = Wk.reshape(DT, P, INNER).transpose(1, 0, 2).astype(F8)
    wv_h = Wv.reshape(DT, P, OUTER).transpose(1, 0, 2).astype(BF)
    bqs_h = bq.reshape(MI, P).T.copy()
    bk_h = bk.reshape(MI, P).T.copy()
    bvb_h = np.broadcast_to(bv, (2 * NSLOTS, OUTER)).copy()
    idr_h = np.eye(P, dtype=np.float32)

    a16 = a_pad.astype(BF)
    b16 = b_pad.astype(BF)

    # ---- per-core inputs ----
    in_maps = []
    for c in range(NCORES):
        nat = np.zeros((tot * P, DIM), BF)
        tr = np.zeros((P, DT, tot * P), F8)
        sm = np.zeros((P, NSM), np.float32)
        gs_a = sm[:, 0:tot_at]
        gs_b = sm[:, tot_at:tot_at + tot_bt]
        npa = sm[:, tot_at + tot_bt:tot_at + tot_bt + NSLOTS]
        npb = sm[:, tot_at + tot_bt + NSLOTS:tot_at + tot_bt + 2 * NSLOTS]
        sm[:, NSM - 2 * MI:NSM - MI] = bqs_h
        sm[:, NSM - MI:NSM] = bk_h
        for s in range(NSLOTS):
            i = groups[s][c]
            la_i, lb_i = int(qa_len[i]), int(qb_len[i])
            A = b16[i] if swap[i] else a16[i]
            Bm = a16[i] if swap[i] else b16[i]
            ao = cum[s] * P                  # A rows/cols at slot start
            bo = (cum[s] + slot_at[s]) * P   # B rows/cols after A's
            nat[ao:ao + la_i] = A[:la_i]
            nat[bo:bo + lb_i] = Bm[:lb_i]
            # transposed layout: [dpart, dt, seq]
            tr[:, :, ao:ao + la_i] = \
                A[:la_i].T.reshape(DT, P, la_i).transpose(1, 0, 2)
            tr[:, :, bo:bo + lb_i] = \
                Bm[:lb_i].T.reshape(DT, P, lb_i).transpose(1, 0, 2)
            # g columns NEGATED (sign trick pairs with den_f = pc - den)
            ga = np.zeros(slot_at[s] * P, np.float32)
            ga[:la_i] = -1.0 / la_i
            gs_a[:, cum_at[s]:cum_at[s] + slot_at[s]] = \
                ga.reshape(slot_at[s], P).T
            gb = np.zeros(slot_bt[s] * P, np.float32)
            gb[:lb_i] = -1.0 / lb_i
            gs_b[:, cum_bt[s]:cum_bt[s] + slot_bt[s]] = \
                gb.reshape(slot_bt[s], P).T
            na_i = slot_at[s] * P - la_i
            nb_i = slot_bt[s] * P - lb_i
            npa[:, s] = np.log(na_i) if na_i > 0 else -1e30
            npb[:, s] = np.log(nb_i) if nb_i > 0 else -1e30
        in_maps.append({
            "nat": nat, "tr": tr, "sm": sm,
            "wq": wq_h, "wk": wk_h, "wv": wv_h,
            "bvb": bvb_h, "idr": idr_h,
        })

    nc = _build_program(slot_at, slot_bt)

    from concourse.bass_utils import run_bass_kernel_spmd

    trace = os.environ.get("BASS_KERNEL_TRACE", "0") == "1"
    if trace:
        _install_profhook()
    res = run_bass_kernel_spmd(nc, in_maps, list(range(NCORES)), trace=trace)
    LAST_EXEC_TIME_NS = res.exec_time_ns

    emb_a = np.zeros((B, OUTER), np.float32)
    emb_b = np.zeros((B, OUTER), np.float32)
    for c in range(NCORES):
        e = np.asarray(res.results[c]["emb"], np.float32)
        for s in range(NSLOTS):
            i = groups[s][c]
            ea, eb = e[2 * s], e[2 * s + 1]  # A-queries, B-queries
            if swap[i]:
                emb_a[i], emb_b[i] = eb, ea
            else:
                emb_a[i], emb_b[i] = ea, eb
    return emb_a, emb_b
